# revision 1
# baseline (speedup 1.0000x reference)
"""GraphTransformerLayer kernel for 8 Trainium2 NeuronCores.

Sharding: 4 graphs per core (batch is sorted, graphs are contiguous).
Each core runs the full layer (QKV proj -> block-diag attention -> out proj)
on its own graphs; cores are fully independent (no collectives).

Device layout is transposed [feature, node] so every matmul maps onto the
PE array without transposes:
  - qT/kT = W @ xT                       [512, nodes]  (per-feature bias via ACT)
  - v' = x @ Wv'  node-major             [nodes, 520]  (8 heads x (64 dims + ones col))
  - sT[j,i] = k_h^T q_h  per (graph, head); pad-key mask + 1/sqrt(d) scale fused
    into ACT Exp via per-partition bias
  - attn@v with denominator appended as column 65 of v' (ones col via bias)
  - normalize: DVE reciprocal of denom row, K=1 matmul broadcast, DVE multiply
  - o-proj per graph, overlapped with later graphs' attention
All matmul inputs are bf16 (full PE rate), fp32 PSUM accumulate; softmax,
normalization, biases, and the final output stay fp32.
"""

import os
import sys

import numpy as np
import ml_dtypes

for _p in ("/opt/trn_rl_repo", "/root/.axon_site/_ro/trn_rl_repo"):
    if os.path.isdir(_p) and _p not in sys.path:
        sys.path.insert(0, _p)

DIM = 512
H = 8
DH = 64
NUM_GRAPHS = 32
N_CORES = 8
GPG = NUM_GRAPHS // N_CORES  # graphs per core
VC = H * (DH + 1)  # 520: v' columns (per head: 64 dims + 1 ones col)
SCALE = 1.0 / np.sqrt(DH)
NEG = -1e30

_NC_CACHE = {}
LAST_RESULTS = None


def _build(GPAD):
    import concourse.bass as bass
    import concourse.tile as tile
    from concourse import mybir
    from contextlib import ExitStack

    f32 = mybir.dt.float32
    b16 = mybir.dt.bfloat16
    AF = mybir.ActivationFunctionType

    JT = -(-GPAD // 128)  # j-tiles per graph (full 128-row tiles, masked)
    JSPAN = JT * 128
    NPAD = GPG * GPAD  # padded node columns per core (q/concat/out)
    NPX = NPAD + (JSPAN - GPAD)  # x/k get extra cols so last graph's j-span exists

    def ntiles(total, cap=512):
        out, off = [], 0
        while off < total:
            w = min(cap, total - off)
            out.append((off, w))
            off += w
        return out

    nc = bass.Bass()
    xT_d = nc.declare_dram_parameter("xT", [DIM, NPX], b16, isOutput=False)
    wq_d = nc.declare_dram_parameter("wqT", [DIM, DIM], b16, isOutput=False)
    wk_d = nc.declare_dram_parameter("wkT", [DIM, DIM], b16, isOutput=False)
    wv_d = nc.declare_dram_parameter("wvT", [DIM, VC], b16, isOutput=False)
    wo_d = nc.declare_dram_parameter("woT", [DIM, DIM], b16, isOutput=False)
    bq_d = nc.declare_dram_parameter("bq", [128, 4], f32, isOutput=False)
    bk_d = nc.declare_dram_parameter("bk", [128, 4], f32, isOutput=False)
    bo_d = nc.declare_dram_parameter("bo", [128, 4], f32, isOutput=False)
    bv_d = nc.declare_dram_parameter("bvrep", [128, VC], b16, isOutput=False)
    mask_d = nc.declare_dram_parameter("mask", [128, GPG * JT], f32, isOutput=False)
    out_d = nc.declare_dram_parameter("outT", [DIM, NPAD], f32, isOutput=True)

    with ExitStack() as ctx:
        tc = ctx.enter_context(tile.TileContext(nc))
        wpool = ctx.enter_context(tc.tile_pool(name="w", bufs=1))
        xpool = ctx.enter_context(tc.tile_pool(name="x", bufs=1))
        apool = ctx.enter_context(tc.tile_pool(name="acts", bufs=1))
        vpool = ctx.enter_context(tc.tile_pool(name="v", bufs=1))
        epool = ctx.enter_context(tc.tile_pool(name="e", bufs=9))
        mpool = ctx.enter_context(tc.tile_pool(name="m", bufs=6))
        opool = ctx.enter_context(tc.tile_pool(name="o", bufs=4))
        ps = ctx.enter_context(tc.tile_pool(name="ps", bufs=4, space="PSUM"))
        avps = ctx.enter_context(tc.tile_pool(name="avps", bufs=2, space="PSUM"))
        bcps = ctx.enter_context(tc.tile_pool(name="bcps", bufs=2, space="PSUM"))

        def load_kb(dram, name, width):
            ts = []
            for kb in range(4):
                t = wpool.tile([128, width], b16, tag=f"{name}{kb}", name=f"{name}{kb}")
                nc.sync.dma_start(t[:], dram[kb * 128:(kb + 1) * 128, :])
                ts.append(t)
            return ts

        wq_sb = load_kb(wq_d, "wq", DIM)
        x_sb = []
        for kb in range(4):
            t = xpool.tile([128, NPX], b16, tag=f"x{kb}", name=f"x{kb}")
            nc.sync.dma_start(t[:], xT_d[kb * 128:(kb + 1) * 128, :])
            x_sb.append(t)
        wk_sb = load_kb(wk_d, "wk", DIM)
        wv_sb = load_kb(wv_d, "wv", VC)
        wo_sb = load_kb(wo_d, "wo", DIM)

        bq_sb = wpool.tile([128, 4], f32, tag="bq")
        nc.sync.dma_start(bq_sb[:], bq_d[:])
        bk_sb = wpool.tile([128, 4], f32, tag="bk")
        nc.sync.dma_start(bk_sb[:], bk_d[:])
        bo_sb = wpool.tile([128, 4], f32, tag="bo")
        nc.sync.dma_start(bo_sb[:], bo_d[:])
        bv_sb = wpool.tile([128, VC], b16, tag="bv")
        nc.sync.dma_start(bv_sb[:], bv_d[:])
        mask_sb = wpool.tile([128, GPG * JT], f32, tag="mask")
        nc.sync.dma_start(mask_sb[:], mask_d[:])
        ones_sb = wpool.tile([1, DH], b16, tag="ones")
        nc.gpsimd.memset(ones_sb[:], 1.0)

        # persistent activations
        qT_sb = [apool.tile([128, NPAD], b16, tag=f"q{fb}", name=f"qT{fb}") for fb in range(4)]
        kT_sb = [apool.tile([128, NPX], b16, tag=f"k{fb}", name=f"kT{fb}") for fb in range(4)]
        cT_sb = [apool.tile([128, NPAD], b16, tag=f"c{fb}", name=f"cT{fb}") for fb in range(4)]

        # ---- q / k projections: out[fb*128+m, n] = sum_k W[m,k] x[n,k] + b[m]
        for (w_sb, b_sb, dst, width) in (
            (wq_sb, bq_sb, qT_sb, NPAD),
            (wk_sb, bk_sb, kT_sb, NPX),
        ):
            for fb in range(4):
                for (off, w) in ntiles(width):
                    p = ps.tile([128, 512], f32, tag="ps", name="psb")
                    for kb in range(4):
                        nc.tensor.matmul(
                            p[:, :w],
                            w_sb[kb][:, fb * 128:(fb + 1) * 128],
                            x_sb[kb][:, off:off + w],
                            start=(kb == 0),
                            stop=(kb == 3),
                        )
                    nc.scalar.activation(
                        dst[fb][:, off:off + w], p[:, :w], AF.Identity,
                        bias=b_sb[:, fb:fb + 1],
                    )

        # ---- v' projection (node-major): v[(g,jb)][j, c] for 128-row j tiles
        v_sb = {}
        for g in range(GPG):
            for jb in range(JT):
                vt = vpool.tile([128, VC], b16, tag=f"v{g}_{jb}", name=f"v{g}_{jb}")
                col0 = g * GPAD + jb * 128
                for (off, w) in ntiles(VC):
                    p = ps.tile([128, 512], f32, tag="ps", name="psb")
                    for kb in range(4):
                        nc.tensor.matmul(
                            p[:, :w],
                            x_sb[kb][:, col0:col0 + 128],
                            wv_sb[kb][:, off:off + w],
                            start=(kb == 0),
                            stop=(kb == 3),
                        )
                    nc.vector.tensor_add(vt[:, off:off + w], p[:, :w],
                                         bv_sb[:, off:off + w])
                v_sb[(g, jb)] = vt

        # ---- attention, software-pipelined by (graph, head) step
        def emit_scores(g, h):
            fb, po = h // 2, 64 * (h % 2)
            etiles = []
            for jb in range(JT):
                jcol = g * GPAD + jb * 128
                sp = ps.tile([128, GPAD], f32, tag="ps", name="sps")
                nc.tensor.matmul(
                    sp[:],
                    kT_sb[fb][po:po + 64, jcol:jcol + 128],
                    qT_sb[fb][po:po + 64, g * GPAD:(g + 1) * GPAD],
                    start=True, stop=True,
                    tile_position=(po, 0),
                )
                et = epool.tile([128, GPAD], b16, tag="e", name="et")
                nc.scalar.activation(
                    et[:], sp[:], AF.Exp,
                    bias=mask_sb[:, g * JT + jb:g * JT + jb + 1],
                    scale=float(SCALE),
                )
                etiles.append(et)
            return etiles

        def emit_attnv(g, h, etiles):
            op = avps.tile([DH + 1, GPAD], f32, tag="av", name="avp")
            for jb in range(JT):
                nc.tensor.matmul(
                    op[:],
                    v_sb[(g, jb)][:, 65 * h:65 * h + 65],
                    etiles[jb][:],
                    start=(jb == 0), stop=(jb == JT - 1),
                )
            rc = mpool.tile([1, GPAD], f32, tag="rc", name="rc")
            nc.vector.reciprocal(rc[:], op[DH:DH + 1, :])
            rc16 = mpool.tile([1, GPAD], b16, tag="rc16", name="rc16")
            nc.vector.tensor_copy(rc16[:], rc[:])
            return op, rc16

        def emit_norm(g, h, op, rc):
            bp = bcps.tile([DH, GPAD], f32, tag="bc", name="bcp")
            nc.tensor.matmul(bp[:], ones_sb[:], rc[:],
                             start=True, stop=True)
            rb = mpool.tile([DH, GPAD], f32, tag="rb", name="rb")
            nc.vector.tensor_copy(rb[:], bp[:])
            fb, po = h // 2, 64 * (h % 2)
            nc.vector.tensor_mul(
                cT_sb[fb][po:po + DH, g * GPAD:(g + 1) * GPAD],
                op[0:DH, :], rb[:],
            )

        def emit_oproj(g):
            for fb in range(4):
                p = ps.tile([128, 512], f32, tag="ps", name="psb")
                for kb in range(4):
                    nc.tensor.matmul(
                        p[:, :GPAD],
                        wo_sb[kb][:, fb * 128:(fb + 1) * 128],
                        cT_sb[kb][:, g * GPAD:(g + 1) * GPAD],
                        start=(kb == 0), stop=(kb == 3),
                    )
                ot = opool.tile([128, GPAD], f32, tag="ot", name="ot")
                nc.scalar.activation(ot[:], p[:, :GPAD], AF.Identity,
                                     bias=bo_sb[:, fb:fb + 1])
                nc.sync.dma_start(
                    out_d[fb * 128:(fb + 1) * 128, g * GPAD:(g + 1) * GPAD], ot[:])

        steps = [(g, h) for g in range(GPG) for h in range(H)]
        S = len(steps)
        pend = {}
        for t in range(S + 3):
            if t < S:
                g, h = steps[t]
                pend[t] = [emit_scores(g, h)]
            if 0 <= t - 2 < S:
                g, h = steps[t - 2]
                op, rc = emit_attnv(g, h, pend[t - 2][0])
                pend[t - 2] += [op, rc]
            if 0 <= t - 3 < S:
                g, h = steps[t - 3]
                _, op, rc = pend.pop(t - 3)
                emit_norm(g, h, op, rc)
                if h == H - 1:
                    emit_oproj(g)

    _split_multiwaits(nc, mybir)
    return nc, dict(GPAD=GPAD, JT=JT, NPAD=NPAD, NPX=NPX)


def _split_multiwaits(nc, mybir, max_waits=1):
    """The pinned walrus codegen accepts only one sync-wait per instruction;
    move extra waits onto dedicated NoOps just before the instruction (same
    engine stream, so semantics are identical)."""
    n_split = 0
    for fn in nc.m.functions:
        for blk in fn.blocks:
            new_insts = []
            for inst in blk.instructions:
                si = getattr(inst, "sync_info", None)
                if si is not None and si.on_wait and len(si.on_wait) > max_waits:
                    waits = list(si.on_wait)
                    extra, keep = waits[:-max_waits], waits[-max_waits:]
                    for i, w in enumerate(extra):
                        new_insts.append(mybir.InstNoOp(
                            name=f"{inst.name}-w{i}",
                            sync_info=mybir.SyncInfo(on_wait=[w], on_update=[]),
                            engine=inst.engine,
                            bass_nofuse=True,
                        ))
                    inst.sync_info = mybir.SyncInfo(on_wait=keep,
                                                    on_update=si.on_update)
                    n_split += 1
                new_insts.append(inst)
            blk.instructions = new_insts
    return n_split


def _get_nc(GPAD):
    if GPAD not in _NC_CACHE:
        _NC_CACHE[GPAD] = _build(GPAD)
    return _NC_CACHE[GPAD]


def kernel(x, batch, w_q, w_k, w_v, b_q, b_k, b_v, w_o, b_o):
    global LAST_RESULTS
    x = np.asarray(x, np.float32)
    batch = np.asarray(batch, np.int64)
    counts = np.bincount(batch, minlength=NUM_GRAPHS)[:NUM_GRAPHS]
    starts = np.concatenate([[0], np.cumsum(counts)]).astype(np.int64)
    GPAD = int(max(256, -(-int(counts.max()) // 64) * 64))
    assert GPAD <= 512, f"graph too large: {counts.max()}"
    nc, meta = _get_nc(GPAD)
    JT, NPAD, NPX = meta["JT"], meta["NPAD"], meta["NPX"]

    # shared host-side prepped weights (bf16 for matmul operands)
    bf16 = ml_dtypes.bfloat16
    wqT = np.ascontiguousarray(w_q.T).astype(bf16)
    wkT = np.ascontiguousarray(w_k.T).astype(bf16)
    woT = np.ascontiguousarray(w_o.T).astype(bf16)
    wvT = np.zeros((DIM, VC), np.float32)
    bvp = np.zeros(VC, np.float32)
    for h in range(H):
        wvT[:, 65 * h:65 * h + 64] = w_v[64 * h:64 * h + 64, :].T
        bvp[65 * h:65 * h + 64] = b_v[64 * h:64 * h + 64]
        bvp[65 * h + 64] = 1.0
    wvT = wvT.astype(bf16)
    bvrep = np.ascontiguousarray(np.broadcast_to(bvp, (128, VC))).astype(bf16)
    bq = np.ascontiguousarray(b_q.reshape(4, 128).T.astype(np.float32))
    bk = np.ascontiguousarray(b_k.reshape(4, 128).T.astype(np.float32))
    bo = np.ascontiguousarray(b_o.reshape(4, 128).T.astype(np.float32))

    in_maps = []
    for c in range(N_CORES):
        xs = np.zeros((NPX, DIM), np.float32)
        mask = np.full((128, GPG * JT), NEG, np.float32)
        for s in range(GPG):
            g = c * GPG + s
            n = int(counts[g])
            xs[s * GPAD:s * GPAD + n] = x[starts[g]:starts[g] + n]
            for jb in range(JT):
                valid = np.arange(128) + jb * 128 < n
                mask[valid, s * JT + jb] = 0.0
        in_maps.append({
            "xT": np.ascontiguousarray(xs.T).astype(bf16),
            "wqT": wqT, "wkT": wkT, "wvT": wvT, "woT": woT,
            "bq": bq, "bk": bk, "bo": bo, "bvrep": bvrep, "mask": mask,
        })

    from concourse.bass_utils import run_bass_kernel_spmd
    trace = bool(os.environ.get("KTRACE"))
    LAST_RESULTS = run_bass_kernel_spmd(nc, in_maps, list(range(N_CORES)),
                                        trace=trace)

    out = np.empty((x.shape[0], DIM), np.float32)
    for c in range(N_CORES):
        oT = LAST_RESULTS.results[c]["outT"]
        for s in range(GPG):
            g = c * GPG + s
            n = int(counts[g])
            out[starts[g]:starts[g] + n] = oT[:, s * GPAD:s * GPAD + n].T
    return out



# revision 50
# speedup vs baseline: 1.0406x; 1.0406x over previous
"""GraphTransformerLayer kernel for 8 Trainium2 NeuronCores.

Sharding: 4 graphs per core (batch is sorted, graphs are contiguous).
Each core runs the full layer (QKV proj -> block-diag attention -> out proj)
on its own graphs; cores are fully independent (no collectives).

Device layout is transposed [feature, node] so every matmul maps onto the
PE array without transposes:
  - qT/kT = W @ xT                       [512, nodes]  (per-feature bias via ACT)
  - v' = x @ Wv'  node-major             [nodes, 520]  (8 heads x (64 dims + ones col))
  - sT[j,i] = k_h^T q_h  per (graph, head); pad-key mask + 1/sqrt(d) scale fused
    into ACT Exp via per-partition bias
  - attn@v with denominator appended as column 65 of v' (ones col via bias)
  - normalize: DVE reciprocal of denom row, K=1 matmul broadcast, DVE multiply
  - o-proj per graph, overlapped with later graphs' attention
All matmul inputs are bf16 (full PE rate), fp32 PSUM accumulate; softmax,
normalization, biases, and the final output stay fp32.
"""

import os
import sys

import numpy as np
import ml_dtypes

for _p in ("/opt/trn_rl_repo", "/root/.axon_site/_ro/trn_rl_repo"):
    if os.path.isdir(_p) and _p not in sys.path:
        sys.path.insert(0, _p)

DIM = 512
H = 8
DH = 64
NUM_GRAPHS = 32
N_CORES = 8
GPG = NUM_GRAPHS // N_CORES  # graphs per core
VC = H * (DH + 1)  # 520: v' columns (per head: 64 dims + 1 ones col)
SCALE = 1.0 / np.sqrt(DH)
NEG = -1e30

_NC_CACHE = {}
LAST_RESULTS = None
LAST_NC = None


def _build(GPAD):
    import concourse.bass as bass
    import concourse.tile as tile
    from concourse import mybir
    from contextlib import ExitStack

    f32 = mybir.dt.float32
    b16 = mybir.dt.bfloat16
    AF = mybir.ActivationFunctionType

    JT = -(-GPAD // 128)  # j-tiles per graph (full 128-row tiles, masked)
    JSPAN = JT * 128
    NPAD = GPG * GPAD  # padded node columns per core (q/concat/out)
    NPX = NPAD + (JSPAN - GPAD)  # x/k get extra cols so last graph's j-span exists

    def ntiles(total, cap=512):
        out, off = [], 0
        while off < total:
            w = min(cap, total - off)
            out.append((off, w))
            off += w
        return out

    nc = bass.Bass()
    xT_d = nc.declare_dram_parameter("xT", [DIM, NPX], b16, isOutput=False)
    wq_d = nc.declare_dram_parameter("wqT", [DIM, DIM], b16, isOutput=False)
    wk_d = nc.declare_dram_parameter("wkT", [DIM, DIM], b16, isOutput=False)
    wv_d = nc.declare_dram_parameter("wvT", [DIM, VC], b16, isOutput=False)
    wo_d = nc.declare_dram_parameter("woT", [DIM, DIM], b16, isOutput=False)
    bq_d = nc.declare_dram_parameter("bq", [128, 4], f32, isOutput=False)
    bk_d = nc.declare_dram_parameter("bk", [128, 4], f32, isOutput=False)
    bo_d = nc.declare_dram_parameter("bo", [128, 4], f32, isOutput=False)
    bv_d = nc.declare_dram_parameter("bvrep", [128, VC], b16, isOutput=False)
    mask_d = nc.declare_dram_parameter("mask", [128, GPG * JT], f32, isOutput=False)
    out_d = nc.declare_dram_parameter("outT", [DIM, NPAD], f32, isOutput=True)

    with ExitStack() as ctx:
        tc = ctx.enter_context(tile.TileContext(nc))
        wpool = ctx.enter_context(tc.tile_pool(name="w", bufs=1))
        xpool = ctx.enter_context(tc.tile_pool(name="x", bufs=1))
        apool = ctx.enter_context(tc.tile_pool(name="acts", bufs=1))
        vpool = ctx.enter_context(tc.tile_pool(name="v", bufs=1))
        epool = ctx.enter_context(tc.tile_pool(name="e", bufs=9))
        mpool = ctx.enter_context(tc.tile_pool(name="m", bufs=6))
        opool = ctx.enter_context(tc.tile_pool(name="o", bufs=4))
        ps = ctx.enter_context(tc.tile_pool(name="ps", bufs=4, space="PSUM"))
        avps = ctx.enter_context(tc.tile_pool(name="avps", bufs=2, space="PSUM"))
        bcps = ctx.enter_context(tc.tile_pool(name="bcps", bufs=2, space="PSUM"))

        def load_kb(dram, name, width):
            ts = []
            for kb in range(4):
                t = wpool.tile([128, width], b16, tag=f"{name}{kb}", name=f"{name}{kb}")
                nc.sync.dma_start(t[:], dram[kb * 128:(kb + 1) * 128, :])
                ts.append(t)
            return ts

        wq_sb = load_kb(wq_d, "wq", DIM)
        x_sb = []
        for kb in range(4):
            t = xpool.tile([128, NPX], b16, tag=f"x{kb}", name=f"x{kb}")
            nc.sync.dma_start(t[:], xT_d[kb * 128:(kb + 1) * 128, :])
            x_sb.append(t)
        wk_sb = load_kb(wk_d, "wk", DIM)
        wv_sb = load_kb(wv_d, "wv", VC)
        wo_sb = load_kb(wo_d, "wo", DIM)

        bq_sb = wpool.tile([128, 4], f32, tag="bq")
        nc.sync.dma_start(bq_sb[:], bq_d[:])
        bk_sb = wpool.tile([128, 4], f32, tag="bk")
        nc.sync.dma_start(bk_sb[:], bk_d[:])
        bo_sb = wpool.tile([128, 4], f32, tag="bo")
        nc.sync.dma_start(bo_sb[:], bo_d[:])
        bv_sb = wpool.tile([128, VC], b16, tag="bv")
        nc.sync.dma_start(bv_sb[:], bv_d[:])
        mask_sb = wpool.tile([128, GPG * JT], f32, tag="mask")
        nc.sync.dma_start(mask_sb[:], mask_d[:])
        ones_sb = wpool.tile([1, DH], b16, tag="ones")
        nc.gpsimd.memset(ones_sb[:], 1.0)

        # persistent activations
        qT_sb = [apool.tile([128, NPAD], b16, tag=f"q{fb}", name=f"qT{fb}") for fb in range(4)]
        kT_sb = [apool.tile([128, NPX], b16, tag=f"k{fb}", name=f"kT{fb}") for fb in range(4)]
        cT_sb = [apool.tile([128, NPAD], b16, tag=f"c{fb}", name=f"cT{fb}") for fb in range(4)]

        # ---- q / k projections: out[fb*128+m, n] = sum_k W[m,k] x[n,k] + b[m]
        for (w_sb, b_sb, dst, width) in (
            (wq_sb, bq_sb, qT_sb, NPAD),
            (wk_sb, bk_sb, kT_sb, NPX),
        ):
            for fb in range(4):
                for (off, w) in ntiles(width):
                    p = ps.tile([128, 512], f32, tag="ps", name="psb")
                    for kb in range(4):
                        nc.tensor.matmul(
                            p[:, :w],
                            w_sb[kb][:, fb * 128:(fb + 1) * 128],
                            x_sb[kb][:, off:off + w],
                            start=(kb == 0),
                            stop=(kb == 3),
                        )
                    nc.scalar.activation(
                        dst[fb][:, off:off + w], p[:, :w], AF.Identity,
                        bias=b_sb[:, fb:fb + 1],
                    )

        # ---- v' projection (node-major): v[(g,jb)][j, c] for 128-row j tiles
        v_sb = {}
        for g in range(GPG):
            for jb in range(JT):
                vt = vpool.tile([128, VC], b16, tag=f"v{g}_{jb}", name=f"v{g}_{jb}")
                col0 = g * GPAD + jb * 128
                for (off, w) in ntiles(VC):
                    p = ps.tile([128, 512], f32, tag="ps", name="psb")
                    for kb in range(4):
                        nc.tensor.matmul(
                            p[:, :w],
                            x_sb[kb][:, col0:col0 + 128],
                            wv_sb[kb][:, off:off + w],
                            start=(kb == 0),
                            stop=(kb == 3),
                        )
                    nc.vector.tensor_add(vt[:, off:off + w], p[:, :w],
                                         bv_sb[:, off:off + w])
                v_sb[(g, jb)] = vt

        # ---- attention, software-pipelined by (graph, head) step
        def emit_scores(g, h):
            fb, po = h // 2, 64 * (h % 2)
            etiles = []
            for jb in range(JT):
                jcol = g * GPAD + jb * 128
                sp = ps.tile([128, GPAD], f32, tag="ps", name="sps")
                nc.tensor.matmul(
                    sp[:],
                    kT_sb[fb][po:po + 64, jcol:jcol + 128],
                    qT_sb[fb][po:po + 64, g * GPAD:(g + 1) * GPAD],
                    start=True, stop=True,
                    tile_position=(po, 0),
                )
                et = epool.tile([128, GPAD], b16, tag="e", name="et")
                nc.scalar.activation(
                    et[:], sp[:], AF.Exp,
                    bias=mask_sb[:, g * JT + jb:g * JT + jb + 1],
                    scale=float(SCALE),
                )
                etiles.append(et)
            return etiles

        def emit_attnv(g, h, etiles):
            op = avps.tile([DH + 1, GPAD], f32, tag="av", name="avp")
            for jb in range(JT):
                nc.tensor.matmul(
                    op[:],
                    v_sb[(g, jb)][:, 65 * h:65 * h + 65],
                    etiles[jb][:],
                    start=(jb == 0), stop=(jb == JT - 1),
                )
            rc = mpool.tile([1, GPAD], f32, tag="rc", name="rc")
            nc.vector.reciprocal(rc[:], op[DH:DH + 1, :])
            rc16 = mpool.tile([1, GPAD], b16, tag="rc16", name="rc16")
            nc.vector.tensor_copy(rc16[:], rc[:])
            return op, rc16

        def emit_norm(g, h, op, rc):
            bp = bcps.tile([DH, GPAD], f32, tag="bc", name="bcp")
            nc.tensor.matmul(bp[:], ones_sb[:], rc[:],
                             start=True, stop=True)
            rb = mpool.tile([DH, GPAD], f32, tag="rb", name="rb")
            nc.vector.tensor_copy(rb[:], bp[:])
            fb, po = h // 2, 64 * (h % 2)
            nc.vector.tensor_mul(
                cT_sb[fb][po:po + DH, g * GPAD:(g + 1) * GPAD],
                op[0:DH, :], rb[:],
            )

        def emit_oproj(g):
            for fb in range(4):
                p = ps.tile([128, 512], f32, tag="ps", name="psb")
                for kb in range(4):
                    nc.tensor.matmul(
                        p[:, :GPAD],
                        wo_sb[kb][:, fb * 128:(fb + 1) * 128],
                        cT_sb[kb][:, g * GPAD:(g + 1) * GPAD],
                        start=(kb == 0), stop=(kb == 3),
                    )
                ot = opool.tile([128, GPAD], f32, tag="ot", name="ot")
                nc.scalar.activation(ot[:], p[:, :GPAD], AF.Identity,
                                     bias=bo_sb[:, fb:fb + 1])
                nc.sync.dma_start(
                    out_d[fb * 128:(fb + 1) * 128, g * GPAD:(g + 1) * GPAD], ot[:])

        steps = [(g, h) for g in range(GPG) for h in range(H)]
        S = len(steps)
        pend = {}
        for t in range(S + 3):
            if t < S:
                g, h = steps[t]
                pend[t] = [emit_scores(g, h)]
            if 0 <= t - 2 < S:
                g, h = steps[t - 2]
                op, rc = emit_attnv(g, h, pend[t - 2][0])
                pend[t - 2] += [op, rc]
            if 0 <= t - 3 < S:
                g, h = steps[t - 3]
                _, op, rc = pend.pop(t - 3)
                emit_norm(g, h, op, rc)
                if h == H - 1:
                    emit_oproj(g)

    _split_multiwaits(nc, mybir)
    return nc, dict(GPAD=GPAD, JT=JT, NPAD=NPAD, NPX=NPX)


def _split_multiwaits(nc, mybir, max_waits=1):
    """The pinned walrus codegen accepts only one sync-wait per instruction;
    move extra waits onto dedicated NoOps just before the instruction (same
    engine stream, so semantics are identical)."""
    n_split = 0
    for fn in nc.m.functions:
        for blk in fn.blocks:
            new_insts = []
            for inst in blk.instructions:
                si = getattr(inst, "sync_info", None)
                if si is not None and si.on_wait and len(si.on_wait) > max_waits:
                    waits = list(si.on_wait)
                    extra, keep = waits[:-max_waits], waits[-max_waits:]
                    for i, w in enumerate(extra):
                        new_insts.append(mybir.InstNoOp(
                            name=f"{inst.name}-w{i}",
                            sync_info=mybir.SyncInfo(on_wait=[w], on_update=[]),
                            engine=inst.engine,
                            bass_nofuse=True,
                        ))
                    inst.sync_info = mybir.SyncInfo(on_wait=keep,
                                                    on_update=si.on_update)
                    n_split += 1
                new_insts.append(inst)
            blk.instructions = new_insts
    return n_split


def _get_nc(GPAD):
    if GPAD not in _NC_CACHE:
        _NC_CACHE[GPAD] = _build(GPAD)
    return _NC_CACHE[GPAD]


def kernel(x, batch, w_q, w_k, w_v, b_q, b_k, b_v, w_o, b_o):
    global LAST_RESULTS, LAST_NC
    x = np.asarray(x, np.float32)
    batch = np.asarray(batch, np.int64)
    counts = np.bincount(batch, minlength=NUM_GRAPHS)[:NUM_GRAPHS]
    starts = np.concatenate([[0], np.cumsum(counts)]).astype(np.int64)
    GPAD = int(max(256, -(-int(counts.max()) // 8) * 8))
    assert GPAD <= 512, f"graph too large: {counts.max()}"
    nc, meta = _get_nc(GPAD)
    LAST_NC = nc
    JT, NPAD, NPX = meta["JT"], meta["NPAD"], meta["NPX"]

    # shared host-side prepped weights (bf16 for matmul operands)
    bf16 = ml_dtypes.bfloat16
    wqT = np.ascontiguousarray(w_q.T).astype(bf16)
    wkT = np.ascontiguousarray(w_k.T).astype(bf16)
    woT = np.ascontiguousarray(w_o.T).astype(bf16)
    wvT = np.zeros((DIM, VC), np.float32)
    bvp = np.zeros(VC, np.float32)
    for h in range(H):
        wvT[:, 65 * h:65 * h + 64] = w_v[64 * h:64 * h + 64, :].T
        bvp[65 * h:65 * h + 64] = b_v[64 * h:64 * h + 64]
        bvp[65 * h + 64] = 1.0
    wvT = wvT.astype(bf16)
    bvrep = np.ascontiguousarray(np.broadcast_to(bvp, (128, VC))).astype(bf16)
    bq = np.ascontiguousarray(b_q.reshape(4, 128).T.astype(np.float32))
    bk = np.ascontiguousarray(b_k.reshape(4, 128).T.astype(np.float32))
    bo = np.ascontiguousarray(b_o.reshape(4, 128).T.astype(np.float32))

    in_maps = []
    for c in range(N_CORES):
        xs = np.zeros((NPX, DIM), np.float32)
        mask = np.full((128, GPG * JT), NEG, np.float32)
        for s in range(GPG):
            g = c * GPG + s
            n = int(counts[g])
            xs[s * GPAD:s * GPAD + n] = x[starts[g]:starts[g] + n]
            for jb in range(JT):
                valid = np.arange(128) + jb * 128 < n
                mask[valid, s * JT + jb] = 0.0
        in_maps.append({
            "xT": np.ascontiguousarray(xs.T).astype(bf16),
            "wqT": wqT, "wkT": wkT, "wvT": wvT, "woT": woT,
            "bq": bq, "bk": bk, "bo": bo, "bvrep": bvrep, "mask": mask,
        })

    from concourse.bass_utils import run_bass_kernel_spmd
    trace = bool(os.environ.get("KTRACE"))
    LAST_RESULTS = run_bass_kernel_spmd(nc, in_maps, list(range(N_CORES)),
                                        trace=trace)

    out = np.empty((x.shape[0], DIM), np.float32)
    for c in range(N_CORES):
        oT = LAST_RESULTS.results[c]["outT"]
        for s in range(GPG):
            g = c * GPG + s
            n = int(counts[g])
            out[starts[g]:starts[g] + n] = oT[:, s * GPAD:s * GPAD + n].T
    return out



# revision 51
# speedup vs baseline: 1.0773x; 1.0353x over previous
"""GraphTransformerLayer kernel for 8 Trainium2 NeuronCores.

Sharding: 4 graphs per core (batch is sorted, graphs are contiguous).
Each core runs the full layer (QKV proj -> block-diag attention -> out proj)
on its own graphs; cores are fully independent (no collectives).

Device layout is transposed [feature, node] so every matmul maps onto the
PE array without transposes:
  - qT/kT = W @ xT                       [512, nodes]  (per-feature bias via ACT)
  - v' = x @ Wv'  node-major             [nodes, 520]  (8 heads x (64 dims + ones col))
  - sT[j,i] = k_h^T q_h  per (graph, head); pad-key mask + 1/sqrt(d) scale fused
    into ACT Exp via per-partition bias
  - attn@v with denominator appended as column 65 of v' (ones col via bias)
  - normalize: DVE reciprocal of denom row, K=1 matmul broadcast, DVE multiply
  - o-proj per graph, overlapped with later graphs' attention
All matmul inputs are bf16 (full PE rate), fp32 PSUM accumulate; softmax,
normalization, biases, and the final output stay fp32.
"""

import os
import sys

import numpy as np
import ml_dtypes

for _p in ("/opt/trn_rl_repo", "/root/.axon_site/_ro/trn_rl_repo"):
    if os.path.isdir(_p) and _p not in sys.path:
        sys.path.insert(0, _p)

DIM = 512
H = 8
DH = 64
NUM_GRAPHS = 32
N_CORES = 8
GPG = NUM_GRAPHS // N_CORES  # graphs per core
VC = H * (DH + 1)  # 520: v' columns (per head: 64 dims + 1 ones col)
SCALE = 1.0 / np.sqrt(DH)
NEG = -1e30

_NC_CACHE = {}
LAST_RESULTS = None
LAST_NC = None


def _build(GPAD):
    import concourse.bass as bass
    import concourse.tile as tile
    from concourse import mybir
    from contextlib import ExitStack

    f32 = mybir.dt.float32
    b16 = mybir.dt.bfloat16
    AF = mybir.ActivationFunctionType

    JT = -(-GPAD // 128)  # j-tiles per graph (full 128-row tiles, masked)
    JSPAN = JT * 128
    NPAD = GPG * GPAD  # padded node columns per core (q/concat/out)
    NPX = NPAD + (JSPAN - GPAD)  # x/k get extra cols so last graph's j-span exists

    def ntiles(total, cap=512):
        out, off = [], 0
        while off < total:
            w = min(cap, total - off)
            out.append((off, w))
            off += w
        return out

    nc = bass.Bass()
    xT_d = nc.declare_dram_parameter("xT", [DIM, NPX], b16, isOutput=False)
    wq_d = nc.declare_dram_parameter("wqT", [DIM, DIM], b16, isOutput=False)
    wk_d = nc.declare_dram_parameter("wkT", [DIM, DIM], b16, isOutput=False)
    wv_d = nc.declare_dram_parameter("wvT", [DIM, VC], b16, isOutput=False)
    wo_d = nc.declare_dram_parameter("woT", [DIM, DIM], b16, isOutput=False)
    bq_d = nc.declare_dram_parameter("bq", [128, 4], f32, isOutput=False)
    bk_d = nc.declare_dram_parameter("bk", [128, 4], f32, isOutput=False)
    bo_d = nc.declare_dram_parameter("bo", [128, 4], f32, isOutput=False)
    bv_d = nc.declare_dram_parameter("bvrep", [128, VC], b16, isOutput=False)
    mask_d = nc.declare_dram_parameter("mask", [128, GPG * JT], f32, isOutput=False)
    out_d = nc.declare_dram_parameter("outT", [DIM, NPAD], f32, isOutput=True)

    with ExitStack() as ctx:
        tc = ctx.enter_context(tile.TileContext(nc))
        wpool = ctx.enter_context(tc.tile_pool(name="w", bufs=1))
        xpool = ctx.enter_context(tc.tile_pool(name="x", bufs=1))
        apool = ctx.enter_context(tc.tile_pool(name="acts", bufs=1))
        vpool = ctx.enter_context(tc.tile_pool(name="v", bufs=1))
        epool = ctx.enter_context(tc.tile_pool(name="e", bufs=9))
        mpool = ctx.enter_context(tc.tile_pool(name="m", bufs=6))
        opool = ctx.enter_context(tc.tile_pool(name="o", bufs=4))
        ps = ctx.enter_context(tc.tile_pool(name="ps", bufs=4, space="PSUM"))
        avps = ctx.enter_context(tc.tile_pool(name="avps", bufs=2, space="PSUM"))
        bcps = ctx.enter_context(tc.tile_pool(name="bcps", bufs=2, space="PSUM"))

        def load_kb(dram, name, width):
            ts = []
            for kb in range(4):
                t = wpool.tile([128, width], b16, tag=f"{name}{kb}", name=f"{name}{kb}")
                nc.sync.dma_start(t[:], dram[kb * 128:(kb + 1) * 128, :])
                ts.append(t)
            return ts

        wq_sb = load_kb(wq_d, "wq", DIM)
        x_sb = []
        for kb in range(4):
            t = xpool.tile([128, NPX], b16, tag=f"x{kb}", name=f"x{kb}")
            nc.sync.dma_start(t[:], xT_d[kb * 128:(kb + 1) * 128, :])
            x_sb.append(t)
        wk_sb = load_kb(wk_d, "wk", DIM)
        wv_sb = load_kb(wv_d, "wv", VC)
        wo_sb = load_kb(wo_d, "wo", DIM)

        bq_sb = wpool.tile([128, 4], f32, tag="bq")
        nc.sync.dma_start(bq_sb[:], bq_d[:])
        bk_sb = wpool.tile([128, 4], f32, tag="bk")
        nc.sync.dma_start(bk_sb[:], bk_d[:])
        bo_sb = wpool.tile([128, 4], f32, tag="bo")
        nc.sync.dma_start(bo_sb[:], bo_d[:])
        bv_sb = wpool.tile([128, VC], b16, tag="bv")
        nc.sync.dma_start(bv_sb[:], bv_d[:])
        mask_sb = wpool.tile([128, GPG * JT], f32, tag="mask")
        nc.sync.dma_start(mask_sb[:], mask_d[:])
        ones_sb = wpool.tile([1, DH], b16, tag="ones")
        nc.gpsimd.memset(ones_sb[:], 1.0)

        # persistent activations
        qT_sb = [apool.tile([128, NPAD], b16, tag=f"q{fb}", name=f"qT{fb}") for fb in range(4)]
        kT_sb = [apool.tile([128, NPX], b16, tag=f"k{fb}", name=f"kT{fb}") for fb in range(4)]
        cT_sb = [apool.tile([128, NPAD], b16, tag=f"c{fb}", name=f"cT{fb}") for fb in range(4)]

        # ---- q / k projections: out[fb*128+m, n] = sum_k W[m,k] x[n,k] + b[m]
        for (w_sb, b_sb, dst, width) in (
            (wq_sb, bq_sb, qT_sb, NPAD),
            (wk_sb, bk_sb, kT_sb, NPX),
        ):
            for fb in range(4):
                for (off, w) in ntiles(width):
                    p = ps.tile([128, 512], f32, tag="ps", name="psb")
                    for kb in range(4):
                        nc.tensor.matmul(
                            p[:, :w],
                            w_sb[kb][:, fb * 128:(fb + 1) * 128],
                            x_sb[kb][:, off:off + w],
                            start=(kb == 0),
                            stop=(kb == 3),
                        )
                    nc.scalar.activation(
                        dst[fb][:, off:off + w], p[:, :w], AF.Identity,
                        bias=b_sb[:, fb:fb + 1],
                    )

        # ---- v' projection (node-major): v[(g,jb)][j, c] for 128-row j tiles
        v_sb = {}
        for g in range(GPG):
            for jb in range(JT):
                vt = vpool.tile([128, VC], b16, tag=f"v{g}_{jb}", name=f"v{g}_{jb}")
                col0 = g * GPAD + jb * 128
                for (off, w) in ntiles(VC):
                    p = ps.tile([128, 512], f32, tag="ps", name="psb")
                    for kb in range(4):
                        nc.tensor.matmul(
                            p[:, :w],
                            x_sb[kb][:, col0:col0 + 128],
                            wv_sb[kb][:, off:off + w],
                            start=(kb == 0),
                            stop=(kb == 3),
                        )
                    nc.vector.tensor_add(vt[:, off:off + w], p[:, :w],
                                         bv_sb[:, off:off + w])
                v_sb[(g, jb)] = vt

        # ---- attention, software-pipelined by (graph, head) step
        def emit_scores(g, h):
            fb, po = h // 2, 64 * (h % 2)
            etiles = []
            for jb in range(JT):
                jcol = g * GPAD + jb * 128
                sp = ps.tile([128, GPAD], f32, tag="ps", name="sps")
                nc.tensor.matmul(
                    sp[:],
                    kT_sb[fb][po:po + 64, jcol:jcol + 128],
                    qT_sb[fb][po:po + 64, g * GPAD:(g + 1) * GPAD],
                    start=True, stop=True,
                    tile_position=(po, 0),
                )
                et = epool.tile([128, GPAD], b16, tag="e", name="et")
                nc.scalar.activation(
                    et[:], sp[:], AF.Exp,
                    bias=mask_sb[:, g * JT + jb:g * JT + jb + 1],
                    scale=float(SCALE),
                )
                etiles.append(et)
            return etiles

        def emit_attnv(g, h, etiles):
            op = avps.tile([DH + 1, GPAD], f32, tag="av", name="avp")
            for jb in range(JT):
                nc.tensor.matmul(
                    op[:],
                    v_sb[(g, jb)][:, 65 * h:65 * h + 65],
                    etiles[jb][:],
                    start=(jb == 0), stop=(jb == JT - 1),
                )
            rc16 = mpool.tile([1, GPAD], b16, tag="rc16", name="rc16")
            with nc.allow_low_precision(reason="bf16 1/denom: same rounding "
                                        "as the old recip->bf16-copy pair"):
                nc.vector.reciprocal(rc16[:], op[DH:DH + 1, :])
            return op, rc16

        def emit_norm(g, h, op, rc):
            bp = bcps.tile([DH, GPAD], f32, tag="bc", name="bcp")
            nc.tensor.matmul(bp[:], ones_sb[:], rc[:],
                             start=True, stop=True)
            rb = mpool.tile([DH, GPAD], f32, tag="rb", name="rb")
            nc.vector.tensor_copy(rb[:], bp[:])
            fb, po = h // 2, 64 * (h % 2)
            nc.vector.tensor_mul(
                cT_sb[fb][po:po + DH, g * GPAD:(g + 1) * GPAD],
                op[0:DH, :], rb[:],
            )

        def emit_oproj(g):
            for fb in range(4):
                p = ps.tile([128, 512], f32, tag="ps", name="psb")
                for kb in range(4):
                    nc.tensor.matmul(
                        p[:, :GPAD],
                        wo_sb[kb][:, fb * 128:(fb + 1) * 128],
                        cT_sb[kb][:, g * GPAD:(g + 1) * GPAD],
                        start=(kb == 0), stop=(kb == 3),
                    )
                ot = opool.tile([128, GPAD], f32, tag="ot", name="ot")
                nc.scalar.activation(ot[:], p[:, :GPAD], AF.Identity,
                                     bias=bo_sb[:, fb:fb + 1])
                nc.sync.dma_start(
                    out_d[fb * 128:(fb + 1) * 128, g * GPAD:(g + 1) * GPAD], ot[:])

        steps = [(g, h) for g in range(GPG) for h in range(H)]
        S = len(steps)
        pend = {}
        for t in range(S + 3):
            if t < S:
                g, h = steps[t]
                pend[t] = [emit_scores(g, h)]
            if 0 <= t - 2 < S:
                g, h = steps[t - 2]
                op, rc = emit_attnv(g, h, pend[t - 2][0])
                pend[t - 2] += [op, rc]
            if 0 <= t - 3 < S:
                g, h = steps[t - 3]
                _, op, rc = pend.pop(t - 3)
                emit_norm(g, h, op, rc)
                if h == H - 1:
                    emit_oproj(g)

    _split_multiwaits(nc, mybir)
    return nc, dict(GPAD=GPAD, JT=JT, NPAD=NPAD, NPX=NPX)


def _split_multiwaits(nc, mybir, max_waits=1):
    """The pinned walrus codegen accepts only one sync-wait per instruction;
    move extra waits onto dedicated NoOps just before the instruction (same
    engine stream, so semantics are identical)."""
    n_split = 0
    for fn in nc.m.functions:
        for blk in fn.blocks:
            new_insts = []
            for inst in blk.instructions:
                si = getattr(inst, "sync_info", None)
                if si is not None and si.on_wait and len(si.on_wait) > max_waits:
                    waits = list(si.on_wait)
                    extra, keep = waits[:-max_waits], waits[-max_waits:]
                    for i, w in enumerate(extra):
                        new_insts.append(mybir.InstNoOp(
                            name=f"{inst.name}-w{i}",
                            sync_info=mybir.SyncInfo(on_wait=[w], on_update=[]),
                            engine=inst.engine,
                            bass_nofuse=True,
                        ))
                    inst.sync_info = mybir.SyncInfo(on_wait=keep,
                                                    on_update=si.on_update)
                    n_split += 1
                new_insts.append(inst)
            blk.instructions = new_insts
    return n_split


def _get_nc(GPAD):
    if GPAD not in _NC_CACHE:
        _NC_CACHE[GPAD] = _build(GPAD)
    return _NC_CACHE[GPAD]


def kernel(x, batch, w_q, w_k, w_v, b_q, b_k, b_v, w_o, b_o):
    global LAST_RESULTS, LAST_NC
    x = np.asarray(x, np.float32)
    batch = np.asarray(batch, np.int64)
    counts = np.bincount(batch, minlength=NUM_GRAPHS)[:NUM_GRAPHS]
    starts = np.concatenate([[0], np.cumsum(counts)]).astype(np.int64)
    GPAD = int(max(256, -(-int(counts.max()) // 8) * 8))
    assert GPAD <= 512, f"graph too large: {counts.max()}"
    nc, meta = _get_nc(GPAD)
    LAST_NC = nc
    JT, NPAD, NPX = meta["JT"], meta["NPAD"], meta["NPX"]

    # shared host-side prepped weights (bf16 for matmul operands)
    bf16 = ml_dtypes.bfloat16
    wqT = np.ascontiguousarray(w_q.T).astype(bf16)
    wkT = np.ascontiguousarray(w_k.T).astype(bf16)
    woT = np.ascontiguousarray(w_o.T).astype(bf16)
    wvT = np.zeros((DIM, VC), np.float32)
    bvp = np.zeros(VC, np.float32)
    for h in range(H):
        wvT[:, 65 * h:65 * h + 64] = w_v[64 * h:64 * h + 64, :].T
        bvp[65 * h:65 * h + 64] = b_v[64 * h:64 * h + 64]
        bvp[65 * h + 64] = 1.0
    wvT = wvT.astype(bf16)
    bvrep = np.ascontiguousarray(np.broadcast_to(bvp, (128, VC))).astype(bf16)
    bq = np.ascontiguousarray(b_q.reshape(4, 128).T.astype(np.float32))
    bk = np.ascontiguousarray(b_k.reshape(4, 128).T.astype(np.float32))
    bo = np.ascontiguousarray(b_o.reshape(4, 128).T.astype(np.float32))

    in_maps = []
    for c in range(N_CORES):
        xs = np.zeros((NPX, DIM), np.float32)
        mask = np.full((128, GPG * JT), NEG, np.float32)
        for s in range(GPG):
            g = c * GPG + s
            n = int(counts[g])
            xs[s * GPAD:s * GPAD + n] = x[starts[g]:starts[g] + n]
            for jb in range(JT):
                valid = np.arange(128) + jb * 128 < n
                mask[valid, s * JT + jb] = 0.0
        in_maps.append({
            "xT": np.ascontiguousarray(xs.T).astype(bf16),
            "wqT": wqT, "wkT": wkT, "wvT": wvT, "woT": woT,
            "bq": bq, "bk": bk, "bo": bo, "bvrep": bvrep, "mask": mask,
        })

    from concourse.bass_utils import run_bass_kernel_spmd
    trace = bool(os.environ.get("KTRACE"))
    LAST_RESULTS = run_bass_kernel_spmd(nc, in_maps, list(range(N_CORES)),
                                        trace=trace)

    out = np.empty((x.shape[0], DIM), np.float32)
    for c in range(N_CORES):
        oT = LAST_RESULTS.results[c]["outT"]
        for s in range(GPG):
            g = c * GPG + s
            n = int(counts[g])
            out[starts[g]:starts[g] + n] = oT[:, s * GPAD:s * GPAD + n].T
    return out



# revision 54
# speedup vs baseline: 1.1517x; 1.0690x over previous
"""GraphTransformerLayer kernel for 8 Trainium2 NeuronCores.

Sharding: 4 graphs per core (batch is sorted, graphs are contiguous).
Each core runs the full layer (QKV proj -> block-diag attention -> out proj)
on its own graphs; cores are fully independent (no collectives).

Device layout is transposed [feature, node] so every matmul maps onto the
PE array without transposes:
  - qT/kT = W @ xT                       [512, nodes]  (per-feature bias via ACT)
  - v' = x @ Wv'  node-major             [nodes, 520]  (8 heads x (64 dims + ones col))
  - sT[j,i] = k_h^T q_h  per (graph, head); pad-key mask + 1/sqrt(d) scale fused
    into ACT Exp via per-partition bias
  - attn@v with denominator appended as column 65 of v' (ones col via bias)
  - normalize: DVE reciprocal of denom row, K=1 matmul broadcast, DVE multiply
  - o-proj per graph, overlapped with later graphs' attention
All matmul inputs are bf16 (full PE rate), fp32 PSUM accumulate; softmax,
normalization, biases, and the final output stay fp32.
"""

import os
import sys

import numpy as np
import ml_dtypes

for _p in ("/opt/trn_rl_repo", "/root/.axon_site/_ro/trn_rl_repo"):
    if os.path.isdir(_p) and _p not in sys.path:
        sys.path.insert(0, _p)

DIM = 512
H = 8
DH = 64
NUM_GRAPHS = 32
N_CORES = 8
GPG = NUM_GRAPHS // N_CORES  # graphs per core
VC = H * (DH + 1)  # 520: v' columns (per head: 64 dims + 1 ones col)
SCALE = 1.0 / np.sqrt(DH)
NEG = -1e30

_NC_CACHE = {}
LAST_RESULTS = None
LAST_NC = None


def _build(GPAD):
    import concourse.bass as bass
    import concourse.tile as tile
    from concourse import mybir
    from contextlib import ExitStack

    f32 = mybir.dt.float32
    b16 = mybir.dt.bfloat16
    AF = mybir.ActivationFunctionType

    JT = -(-GPAD // 128)  # j-tiles per graph (full 128-row tiles, masked)
    JSPAN = JT * 128
    NPAD = GPG * GPAD  # padded node columns per core (q/concat/out)
    NPX = NPAD + (JSPAN - GPAD)  # x/k get extra cols so last graph's j-span exists

    def ntiles(total, cap=512):
        out, off = [], 0
        while off < total:
            w = min(cap, total - off)
            out.append((off, w))
            off += w
        return out

    # weight blob (wq|wk|wv|wo): 4 DMAs instead of 16; per-DMA issue costs
    # ~1.2us serialized on HWDGE + the SP sequencer, so fewer, bigger DMAs
    # shorten the prologue
    WB = 3 * DIM + VC  # 2056
    MC = 12 + GPG * JT  # misc fp32 columns: bq|bk|bo|mask
    nc = bass.Bass()
    xT_d = nc.declare_dram_parameter("xT", [DIM, NPX], b16, isOutput=False)
    wb_d = nc.declare_dram_parameter("wb", [DIM, WB], b16, isOutput=False)
    bv_d = nc.declare_dram_parameter("bvrep", [128, VC], b16, isOutput=False)
    misc_d = nc.declare_dram_parameter("misc", [128, MC], f32, isOutput=False)
    out_d = nc.declare_dram_parameter("outT", [DIM, NPAD], f32, isOutput=True)

    with ExitStack() as ctx:
        tc = ctx.enter_context(tile.TileContext(nc))
        wpool = ctx.enter_context(tc.tile_pool(name="w", bufs=1))
        xpool = ctx.enter_context(tc.tile_pool(name="x", bufs=1))
        apool = ctx.enter_context(tc.tile_pool(name="acts", bufs=1))
        vpool = ctx.enter_context(tc.tile_pool(name="v", bufs=1))
        epool = ctx.enter_context(tc.tile_pool(name="e", bufs=9))
        mpool = ctx.enter_context(tc.tile_pool(name="m", bufs=6))
        opool = ctx.enter_context(tc.tile_pool(name="o", bufs=4))
        ps = ctx.enter_context(tc.tile_pool(name="ps", bufs=4, space="PSUM"))
        avps = ctx.enter_context(tc.tile_pool(name="avps", bufs=2, space="PSUM"))
        bcps = ctx.enter_context(tc.tile_pool(name="bcps", bufs=2, space="PSUM"))

        # interleave weight-blob and x kb-blocks; tiny misc (biases+mask)
        # right after the first pair so early ACT movers never stall on it
        wb_sb, x_sb = [], []
        misc_sb = bv_sb = None
        for kb in range(4):
            w = wpool.tile([128, WB], b16, tag=f"wb{kb}", name=f"wb{kb}")
            nc.sync.dma_start(w[:], wb_d[kb * 128:(kb + 1) * 128, :])
            wb_sb.append(w)
            t = xpool.tile([128, NPX], b16, tag=f"x{kb}", name=f"x{kb}")
            nc.sync.dma_start(t[:], xT_d[kb * 128:(kb + 1) * 128, :])
            x_sb.append(t)
            if kb == 0:
                misc_sb = wpool.tile([128, MC], f32, tag="misc")
                nc.sync.dma_start(misc_sb[:], misc_d[:])
                bv_sb = wpool.tile([128, VC], b16, tag="bv")
                nc.sync.dma_start(bv_sb[:], bv_d[:])
        wq_sb = [w[:, 0:DIM] for w in wb_sb]
        wk_sb = [w[:, DIM:2 * DIM] for w in wb_sb]
        wv_sb = [w[:, 2 * DIM:2 * DIM + VC] for w in wb_sb]
        wo_sb = [w[:, 2 * DIM + VC:WB] for w in wb_sb]
        bq_sb = misc_sb[:, 0:4]
        bk_sb = misc_sb[:, 4:8]
        bo_sb = misc_sb[:, 8:12]
        mask_sb = misc_sb[:, 12:MC]
        ones_sb = wpool.tile([1, DH], b16, tag="ones")
        nc.gpsimd.memset(ones_sb[:], 1.0)

        # persistent activations
        qT_sb = [apool.tile([128, NPAD], b16, tag=f"q{fb}", name=f"qT{fb}") for fb in range(4)]
        kT_sb = [apool.tile([128, NPX], b16, tag=f"k{fb}", name=f"kT{fb}") for fb in range(4)]
        cT_sb = [apool.tile([128, NPAD], b16, tag=f"c{fb}", name=f"cT{fb}") for fb in range(4)]

        # ---- q / k projections: out[fb*128+m, n] = sum_k W[m,k] x[n,k] + b[m]
        for (w_sb, b_sb, dst, width) in (
            (wq_sb, bq_sb, qT_sb, NPAD),
            (wk_sb, bk_sb, kT_sb, NPX),
        ):
            for fb in range(4):
                for (off, w) in ntiles(width):
                    p = ps.tile([128, 512], f32, tag="ps", name="psb")
                    for kb in range(4):
                        nc.tensor.matmul(
                            p[:, :w],
                            w_sb[kb][:, fb * 128:(fb + 1) * 128],
                            x_sb[kb][:, off:off + w],
                            start=(kb == 0),
                            stop=(kb == 3),
                        )
                    nc.scalar.activation(
                        dst[fb][:, off:off + w], p[:, :w], AF.Identity,
                        bias=b_sb[:, fb:fb + 1],
                    )

        # ---- v' projection (node-major): v[(g,jb)][j, c] for 128-row j tiles
        v_sb = {}
        for g in range(GPG):
            for jb in range(JT):
                vt = vpool.tile([128, VC], b16, tag=f"v{g}_{jb}", name=f"v{g}_{jb}")
                col0 = g * GPAD + jb * 128
                for (off, w) in ntiles(VC):
                    p = ps.tile([128, 512], f32, tag="ps", name="psb")
                    for kb in range(4):
                        nc.tensor.matmul(
                            p[:, :w],
                            x_sb[kb][:, col0:col0 + 128],
                            wv_sb[kb][:, off:off + w],
                            start=(kb == 0),
                            stop=(kb == 3),
                        )
                    nc.vector.tensor_add(vt[:, off:off + w], p[:, :w],
                                         bv_sb[:, off:off + w])
                v_sb[(g, jb)] = vt

        # ---- attention, software-pipelined by (graph, head) step
        def emit_scores(g, h):
            fb, po = h // 2, 64 * (h % 2)
            etiles = []
            for jb in range(JT):
                jcol = g * GPAD + jb * 128
                sp = ps.tile([128, GPAD], f32, tag="ps", name="sps")
                nc.tensor.matmul(
                    sp[:],
                    kT_sb[fb][po:po + 64, jcol:jcol + 128],
                    qT_sb[fb][po:po + 64, g * GPAD:(g + 1) * GPAD],
                    start=True, stop=True,
                    tile_position=(po, 0),
                )
                et = epool.tile([128, GPAD], b16, tag="e", name="et")
                nc.scalar.activation(
                    et[:], sp[:], AF.Exp,
                    bias=mask_sb[:, g * JT + jb:g * JT + jb + 1],
                    scale=float(SCALE),
                )
                etiles.append(et)
            return etiles

        def emit_attnv(g, h, etiles):
            op = avps.tile([DH + 1, GPAD], f32, tag="av", name="avp")
            for jb in range(JT):
                nc.tensor.matmul(
                    op[:],
                    v_sb[(g, jb)][:, 65 * h:65 * h + 65],
                    etiles[jb][:],
                    start=(jb == 0), stop=(jb == JT - 1),
                )
            rc16 = mpool.tile([1, GPAD], b16, tag="rc16", name="rc16")
            with nc.allow_low_precision(reason="bf16 1/denom: same rounding "
                                        "as the old recip->bf16-copy pair"):
                nc.vector.reciprocal(rc16[:], op[DH:DH + 1, :])
            return op, rc16

        def emit_norm(g, h, op, rc):
            bp = bcps.tile([DH, GPAD], f32, tag="bc", name="bcp")
            nc.tensor.matmul(bp[:], ones_sb[:], rc[:],
                             start=True, stop=True)
            rb = mpool.tile([DH, GPAD], f32, tag="rb", name="rb")
            nc.vector.tensor_copy(rb[:], bp[:])
            fb, po = h // 2, 64 * (h % 2)
            nc.vector.tensor_mul(
                cT_sb[fb][po:po + DH, g * GPAD:(g + 1) * GPAD],
                op[0:DH, :], rb[:],
            )

        def emit_oproj(g):
            for fb in range(4):
                p = ps.tile([128, 512], f32, tag="ps", name="psb")
                for kb in range(4):
                    nc.tensor.matmul(
                        p[:, :GPAD],
                        wo_sb[kb][:, fb * 128:(fb + 1) * 128],
                        cT_sb[kb][:, g * GPAD:(g + 1) * GPAD],
                        start=(kb == 0), stop=(kb == 3),
                    )
                ot = opool.tile([128, GPAD], f32, tag="ot", name="ot")
                nc.scalar.activation(ot[:], p[:, :GPAD], AF.Identity,
                                     bias=bo_sb[:, fb:fb + 1])
                nc.sync.dma_start(
                    out_d[fb * 128:(fb + 1) * 128, g * GPAD:(g + 1) * GPAD], ot[:])

        steps = [(g, h) for g in range(GPG) for h in range(H)]
        S = len(steps)
        pend = {}
        for t in range(S + 3):
            if t < S:
                g, h = steps[t]
                pend[t] = [emit_scores(g, h)]
            if 0 <= t - 2 < S:
                g, h = steps[t - 2]
                op, rc = emit_attnv(g, h, pend[t - 2][0])
                pend[t - 2] += [op, rc]
            if 0 <= t - 3 < S:
                g, h = steps[t - 3]
                _, op, rc = pend.pop(t - 3)
                emit_norm(g, h, op, rc)
                if h == H - 1:
                    emit_oproj(g)

    _split_multiwaits(nc, mybir)
    return nc, dict(GPAD=GPAD, JT=JT, NPAD=NPAD, NPX=NPX)


def _split_multiwaits(nc, mybir, max_waits=1):
    """The pinned walrus codegen accepts only one sync-wait per instruction;
    move extra waits onto dedicated NoOps just before the instruction (same
    engine stream, so semantics are identical)."""
    n_split = 0
    for fn in nc.m.functions:
        for blk in fn.blocks:
            new_insts = []
            for inst in blk.instructions:
                si = getattr(inst, "sync_info", None)
                if si is not None and si.on_wait and len(si.on_wait) > max_waits:
                    waits = list(si.on_wait)
                    extra, keep = waits[:-max_waits], waits[-max_waits:]
                    for i, w in enumerate(extra):
                        new_insts.append(mybir.InstNoOp(
                            name=f"{inst.name}-w{i}",
                            sync_info=mybir.SyncInfo(on_wait=[w], on_update=[]),
                            engine=inst.engine,
                            bass_nofuse=True,
                        ))
                    inst.sync_info = mybir.SyncInfo(on_wait=keep,
                                                    on_update=si.on_update)
                    n_split += 1
                new_insts.append(inst)
            blk.instructions = new_insts
    return n_split


def _get_nc(GPAD):
    if GPAD not in _NC_CACHE:
        _NC_CACHE[GPAD] = _build(GPAD)
    return _NC_CACHE[GPAD]


def kernel(x, batch, w_q, w_k, w_v, b_q, b_k, b_v, w_o, b_o):
    global LAST_RESULTS, LAST_NC
    x = np.asarray(x, np.float32)
    batch = np.asarray(batch, np.int64)
    counts = np.bincount(batch, minlength=NUM_GRAPHS)[:NUM_GRAPHS]
    starts = np.concatenate([[0], np.cumsum(counts)]).astype(np.int64)
    GPAD = int(max(256, -(-int(counts.max()) // 8) * 8))
    assert GPAD <= 512, f"graph too large: {counts.max()}"
    nc, meta = _get_nc(GPAD)
    LAST_NC = nc
    JT, NPAD, NPX = meta["JT"], meta["NPAD"], meta["NPX"]

    # shared host-side prepped weights (bf16 for matmul operands)
    bf16 = ml_dtypes.bfloat16
    wvT = np.zeros((DIM, VC), np.float32)
    bvp = np.zeros(VC, np.float32)
    for h in range(H):
        wvT[:, 65 * h:65 * h + 64] = w_v[64 * h:64 * h + 64, :].T
        bvp[65 * h:65 * h + 64] = b_v[64 * h:64 * h + 64]
        bvp[65 * h + 64] = 1.0
    wb = np.concatenate([w_q.T, w_k.T, wvT, w_o.T], axis=1)
    wb = np.ascontiguousarray(wb).astype(bf16)
    bvrep = np.ascontiguousarray(np.broadcast_to(bvp, (128, VC))).astype(bf16)
    misc0 = np.empty((128, 12 + GPG * JT), np.float32)
    misc0[:, 0:4] = b_q.reshape(4, 128).T
    misc0[:, 4:8] = b_k.reshape(4, 128).T
    misc0[:, 8:12] = b_o.reshape(4, 128).T

    in_maps = []
    for c in range(N_CORES):
        xs = np.zeros((NPX, DIM), np.float32)
        misc = misc0.copy()
        misc[:, 12:] = NEG
        for s in range(GPG):
            g = c * GPG + s
            n = int(counts[g])
            xs[s * GPAD:s * GPAD + n] = x[starts[g]:starts[g] + n]
            for jb in range(JT):
                valid = np.arange(128) + jb * 128 < n
                misc[valid, 12 + s * JT + jb] = 0.0
        in_maps.append({
            "xT": np.ascontiguousarray(xs.T).astype(bf16),
            "wb": wb, "bvrep": bvrep, "misc": misc,
        })

    from concourse.bass_utils import run_bass_kernel_spmd
    trace = bool(os.environ.get("KTRACE"))
    LAST_RESULTS = run_bass_kernel_spmd(nc, in_maps, list(range(N_CORES)),
                                        trace=trace)

    out = np.empty((x.shape[0], DIM), np.float32)
    for c in range(N_CORES):
        oT = LAST_RESULTS.results[c]["outT"]
        for s in range(GPG):
            g = c * GPG + s
            n = int(counts[g])
            out[starts[g]:starts[g] + n] = oT[:, s * GPAD:s * GPAD + n].T
    return out



# revision 59
# speedup vs baseline: 1.1702x; 1.0161x over previous
"""GraphTransformerLayer kernel for 8 Trainium2 NeuronCores.

Sharding: 4 graphs per core (batch is sorted, graphs are contiguous).
Each core runs the full layer (QKV proj -> block-diag attention -> out proj)
on its own graphs; cores are fully independent (no collectives).

Device layout is transposed [feature, node] so every matmul maps onto the
PE array without transposes:
  - qT/kT = W @ xT                       [512, nodes]  (per-feature bias via ACT)
  - v' = x @ Wv'  node-major             [nodes, 520]  (8 heads x (64 dims + ones col))
  - sT[j,i] = k_h^T q_h  per (graph, head); pad-key mask + 1/sqrt(d) scale fused
    into ACT Exp via per-partition bias
  - attn@v with denominator appended as column 65 of v' (ones col via bias)
  - normalize: DVE reciprocal of denom row, K=1 matmul broadcast, DVE multiply
  - o-proj per graph, overlapped with later graphs' attention
All matmul inputs are bf16 (full PE rate), fp32 PSUM accumulate; softmax,
normalization, biases, and the final output stay fp32.
"""

import os
import sys

import numpy as np
import ml_dtypes

for _p in ("/opt/trn_rl_repo", "/root/.axon_site/_ro/trn_rl_repo"):
    if os.path.isdir(_p) and _p not in sys.path:
        sys.path.insert(0, _p)

DIM = 512
H = 8
DH = 64
NUM_GRAPHS = 32
N_CORES = 8
GPG = NUM_GRAPHS // N_CORES  # graphs per core
VC = H * (DH + 1)  # 520: v' columns (per head: 64 dims + 1 ones col)
SCALE = 1.0 / np.sqrt(DH)
NEG = -1e30

_NC_CACHE = {}
LAST_RESULTS = None
LAST_NC = None


def _build(GPAD):
    import concourse.bass as bass
    import concourse.tile as tile
    from concourse import mybir
    from contextlib import ExitStack

    f32 = mybir.dt.float32
    b16 = mybir.dt.bfloat16
    AF = mybir.ActivationFunctionType

    JT = -(-GPAD // 128)  # j-tiles per graph (full 128-row tiles, masked)
    JSPAN = JT * 128
    NPAD = GPG * GPAD  # padded node columns per core (q/concat/out)
    NPX = NPAD + (JSPAN - GPAD)  # x/k get extra cols so last graph's j-span exists

    def ntiles(total, cap=512):
        out, off = [], 0
        while off < total:
            w = min(cap, total - off)
            out.append((off, w))
            off += w
        return out

    # weight blob (wq|wk|wv|wo): 4 DMAs instead of 16; per-DMA issue costs
    # ~1.2us serialized on HWDGE + the SP sequencer, so fewer, bigger DMAs
    # shorten the prologue
    WB = 2 * DIM + VC  # 1544: wq|wk|wv; wo ships separately (not needed
    # until the first o-proj, so it must not gate the prologue)
    MC = 12 + GPG * JT  # misc fp32 columns: bq|bk|bo|mask
    nc = bass.Bass()
    xT_d = nc.declare_dram_parameter("xT", [DIM, NPX], b16, isOutput=False)
    wb_d = nc.declare_dram_parameter("wb", [DIM, WB], b16, isOutput=False)
    wo4_d = nc.declare_dram_parameter("wo4", [128, 4 * DIM], b16, isOutput=False)
    bv_d = nc.declare_dram_parameter("bvrep", [128, VC], b16, isOutput=False)
    misc_d = nc.declare_dram_parameter("misc", [128, MC], f32, isOutput=False)
    out_d = nc.declare_dram_parameter("outT", [DIM, NPAD], b16, isOutput=True)

    with ExitStack() as ctx:
        tc = ctx.enter_context(tile.TileContext(nc))
        wpool = ctx.enter_context(tc.tile_pool(name="w", bufs=1))
        xpool = ctx.enter_context(tc.tile_pool(name="x", bufs=1))
        apool = ctx.enter_context(tc.tile_pool(name="acts", bufs=1))
        vpool = ctx.enter_context(tc.tile_pool(name="v", bufs=1))
        epool = ctx.enter_context(tc.tile_pool(name="e", bufs=9))
        mpool = ctx.enter_context(tc.tile_pool(name="m", bufs=6))
        opool = ctx.enter_context(tc.tile_pool(name="o", bufs=4))
        ps = ctx.enter_context(tc.tile_pool(name="ps", bufs=4, space="PSUM"))
        avps = ctx.enter_context(tc.tile_pool(name="avps", bufs=2, space="PSUM"))
        bcps = ctx.enter_context(tc.tile_pool(name="bcps", bufs=2, space="PSUM"))

        # interleave weight-blob and x kb-blocks; tiny misc (biases+mask)
        # right after the first pair so early ACT movers never stall on it
        wb_sb, x_sb = [], []
        misc_sb = bv_sb = None
        for kb in range(4):
            w = wpool.tile([128, WB], b16, tag=f"wb{kb}", name=f"wb{kb}")
            nc.sync.dma_start(w[:], wb_d[kb * 128:(kb + 1) * 128, :])
            wb_sb.append(w)
            t = xpool.tile([128, NPX], b16, tag=f"x{kb}", name=f"x{kb}")
            nc.sync.dma_start(t[:], xT_d[kb * 128:(kb + 1) * 128, :])
            x_sb.append(t)
            if kb == 0:
                misc_sb = wpool.tile([128, MC], f32, tag="misc")
                nc.sync.dma_start(misc_sb[:], misc_d[:])
                bv_sb = wpool.tile([128, VC], b16, tag="bv")
                nc.sync.dma_start(bv_sb[:], bv_d[:])
        wo4_sb = wpool.tile([128, 4 * DIM], b16, tag="wo4")
        nc.sync.dma_start(wo4_sb[:], wo4_d[:])
        wq_sb = [w[:, 0:DIM] for w in wb_sb]
        wk_sb = [w[:, DIM:2 * DIM] for w in wb_sb]
        wv_sb = [w[:, 2 * DIM:2 * DIM + VC] for w in wb_sb]
        wo_sb = [wo4_sb[:, kb * DIM:(kb + 1) * DIM] for kb in range(4)]
        bq_sb = misc_sb[:, 0:4]
        bk_sb = misc_sb[:, 4:8]
        bo_sb = misc_sb[:, 8:12]
        mask_sb = misc_sb[:, 12:MC]
        ones_sb = wpool.tile([1, DH], b16, tag="ones")
        nc.gpsimd.memset(ones_sb[:], 1.0)

        # persistent activations
        qT_sb = [apool.tile([128, NPAD], b16, tag=f"q{fb}", name=f"qT{fb}") for fb in range(4)]
        kT_sb = [apool.tile([128, NPX], b16, tag=f"k{fb}", name=f"kT{fb}") for fb in range(4)]
        cT_sb = [apool.tile([128, NPAD], b16, tag=f"c{fb}", name=f"cT{fb}") for fb in range(4)]

        # ---- q / k projections: out[fb*128+m, n] = sum_k W[m,k] x[n,k] + b[m]
        for (w_sb, b_sb, dst, width) in (
            (wq_sb, bq_sb, qT_sb, NPAD),
            (wk_sb, bk_sb, kT_sb, NPX),
        ):
            for fb in range(4):
                for (off, w) in ntiles(width):
                    p = ps.tile([128, 512], f32, tag="ps", name="psb")
                    for kb in range(4):
                        nc.tensor.matmul(
                            p[:, :w],
                            w_sb[kb][:, fb * 128:(fb + 1) * 128],
                            x_sb[kb][:, off:off + w],
                            start=(kb == 0),
                            stop=(kb == 3),
                        )
                    nc.scalar.activation(
                        dst[fb][:, off:off + w], p[:, :w], AF.Identity,
                        bias=b_sb[:, fb:fb + 1],
                    )

        # ---- v' projection (node-major): v[(g,jb)][j, c] for 128-row j tiles
        v_sb = {}
        for g in range(GPG):
            for jb in range(JT):
                vt = vpool.tile([128, VC], b16, tag=f"v{g}_{jb}", name=f"v{g}_{jb}")
                col0 = g * GPAD + jb * 128
                for (off, w) in ntiles(VC):
                    p = ps.tile([128, 512], f32, tag="ps", name="psb")
                    for kb in range(4):
                        nc.tensor.matmul(
                            p[:, :w],
                            x_sb[kb][:, col0:col0 + 128],
                            wv_sb[kb][:, off:off + w],
                            start=(kb == 0),
                            stop=(kb == 3),
                        )
                    nc.vector.tensor_add(vt[:, off:off + w], p[:, :w],
                                         bv_sb[:, off:off + w])
                v_sb[(g, jb)] = vt

        # ---- attention, software-pipelined by (graph, head) step
        def emit_scores(g, h):
            fb, po = h // 2, 64 * (h % 2)
            etiles = []
            for jb in range(JT):
                jcol = g * GPAD + jb * 128
                sp = ps.tile([128, GPAD], f32, tag="ps", name="sps")
                nc.tensor.matmul(
                    sp[:],
                    kT_sb[fb][po:po + 64, jcol:jcol + 128],
                    qT_sb[fb][po:po + 64, g * GPAD:(g + 1) * GPAD],
                    start=True, stop=True,
                    tile_position=(po, 0),
                )
                et = epool.tile([128, GPAD], b16, tag="e", name="et")
                nc.scalar.activation(
                    et[:], sp[:], AF.Exp,
                    bias=mask_sb[:, g * JT + jb:g * JT + jb + 1],
                    scale=float(SCALE),
                )
                etiles.append(et)
            return etiles

        def emit_attnv(g, h, etiles):
            op = avps.tile([DH + 1, GPAD], f32, tag="av", name="avp")
            for jb in range(JT):
                nc.tensor.matmul(
                    op[:],
                    v_sb[(g, jb)][:, 65 * h:65 * h + 65],
                    etiles[jb][:],
                    start=(jb == 0), stop=(jb == JT - 1),
                )
            rc16 = mpool.tile([1, GPAD], b16, tag="rc16", name="rc16")
            with nc.allow_low_precision(reason="bf16 1/denom: same rounding "
                                        "as the old recip->bf16-copy pair"):
                nc.vector.reciprocal(rc16[:], op[DH:DH + 1, :])
            return op, rc16

        def emit_norm(g, h, op, rc):
            bp = bcps.tile([DH, GPAD], f32, tag="bc", name="bcp")
            nc.tensor.matmul(bp[:], ones_sb[:], rc[:],
                             start=True, stop=True)
            rb = mpool.tile([DH, GPAD], f32, tag="rb", name="rb")
            nc.vector.tensor_copy(rb[:], bp[:])
            fb, po = h // 2, 64 * (h % 2)
            nc.vector.tensor_mul(
                cT_sb[fb][po:po + DH, g * GPAD:(g + 1) * GPAD],
                op[0:DH, :], rb[:],
            )

        def emit_oproj(g):
            for fb in range(4):
                p = ps.tile([128, 512], f32, tag="ps", name="psb")
                for kb in range(4):
                    nc.tensor.matmul(
                        p[:, :GPAD],
                        wo_sb[kb][:, fb * 128:(fb + 1) * 128],
                        cT_sb[kb][:, g * GPAD:(g + 1) * GPAD],
                        start=(kb == 0), stop=(kb == 3),
                    )
                ot = opool.tile([128, GPAD], b16, tag="ot", name="ot")
                nc.scalar.activation(ot[:], p[:, :GPAD], AF.Identity,
                                     bias=bo_sb[:, fb:fb + 1])
                nc.sync.dma_start(
                    out_d[fb * 128:(fb + 1) * 128, g * GPAD:(g + 1) * GPAD], ot[:])

        steps = [(g, h) for g in range(GPG) for h in range(H)]
        S = len(steps)
        pend = {}
        for t in range(S + 3):
            if t < S:
                g, h = steps[t]
                pend[t] = [emit_scores(g, h)]
            if 0 <= t - 2 < S:
                g, h = steps[t - 2]
                op, rc = emit_attnv(g, h, pend[t - 2][0])
                pend[t - 2] += [op, rc]
            if 0 <= t - 3 < S:
                g, h = steps[t - 3]
                _, op, rc = pend.pop(t - 3)
                emit_norm(g, h, op, rc)
                if h == H - 1:
                    emit_oproj(g)

    _split_multiwaits(nc, mybir)
    return nc, dict(GPAD=GPAD, JT=JT, NPAD=NPAD, NPX=NPX)


def _split_multiwaits(nc, mybir, max_waits=1):
    """The pinned walrus codegen accepts only one sync-wait per instruction;
    move extra waits onto dedicated NoOps just before the instruction (same
    engine stream, so semantics are identical)."""
    n_split = 0
    for fn in nc.m.functions:
        for blk in fn.blocks:
            new_insts = []
            for inst in blk.instructions:
                si = getattr(inst, "sync_info", None)
                if si is not None and si.on_wait and len(si.on_wait) > max_waits:
                    waits = list(si.on_wait)
                    extra, keep = waits[:-max_waits], waits[-max_waits:]
                    for i, w in enumerate(extra):
                        new_insts.append(mybir.InstNoOp(
                            name=f"{inst.name}-w{i}",
                            sync_info=mybir.SyncInfo(on_wait=[w], on_update=[]),
                            engine=inst.engine,
                            bass_nofuse=True,
                        ))
                    inst.sync_info = mybir.SyncInfo(on_wait=keep,
                                                    on_update=si.on_update)
                    n_split += 1
                new_insts.append(inst)
            blk.instructions = new_insts
    return n_split


def _get_nc(GPAD):
    if GPAD not in _NC_CACHE:
        _NC_CACHE[GPAD] = _build(GPAD)
    return _NC_CACHE[GPAD]


def kernel(x, batch, w_q, w_k, w_v, b_q, b_k, b_v, w_o, b_o):
    global LAST_RESULTS, LAST_NC
    x = np.asarray(x, np.float32)
    batch = np.asarray(batch, np.int64)
    counts = np.bincount(batch, minlength=NUM_GRAPHS)[:NUM_GRAPHS]
    starts = np.concatenate([[0], np.cumsum(counts)]).astype(np.int64)
    GPAD = int(max(256, -(-int(counts.max()) // 8) * 8))
    assert GPAD <= 512, f"graph too large: {counts.max()}"
    nc, meta = _get_nc(GPAD)
    LAST_NC = nc
    JT, NPAD, NPX = meta["JT"], meta["NPAD"], meta["NPX"]

    # shared host-side prepped weights (bf16 for matmul operands)
    bf16 = ml_dtypes.bfloat16
    wvT = np.zeros((DIM, VC), np.float32)
    bvp = np.zeros(VC, np.float32)
    for h in range(H):
        wvT[:, 65 * h:65 * h + 64] = w_v[64 * h:64 * h + 64, :].T
        bvp[65 * h:65 * h + 64] = b_v[64 * h:64 * h + 64]
        bvp[65 * h + 64] = 1.0
    wb = np.concatenate([w_q.T, w_k.T, wvT], axis=1)
    wb = np.ascontiguousarray(wb).astype(bf16)
    woT = w_o.T
    wo4 = np.concatenate([woT[kb * 128:(kb + 1) * 128] for kb in range(4)],
                         axis=1)
    wo4 = np.ascontiguousarray(wo4).astype(bf16)
    bvrep = np.ascontiguousarray(np.broadcast_to(bvp, (128, VC))).astype(bf16)
    misc0 = np.empty((128, 12 + GPG * JT), np.float32)
    misc0[:, 0:4] = b_q.reshape(4, 128).T
    misc0[:, 4:8] = b_k.reshape(4, 128).T
    misc0[:, 8:12] = b_o.reshape(4, 128).T

    in_maps = []
    for c in range(N_CORES):
        xs = np.zeros((NPX, DIM), np.float32)
        misc = misc0.copy()
        misc[:, 12:] = NEG
        for s in range(GPG):
            g = c * GPG + s
            n = int(counts[g])
            xs[s * GPAD:s * GPAD + n] = x[starts[g]:starts[g] + n]
            for jb in range(JT):
                valid = np.arange(128) + jb * 128 < n
                misc[valid, 12 + s * JT + jb] = 0.0
        in_maps.append({
            "xT": np.ascontiguousarray(xs.T).astype(bf16),
            "wb": wb, "wo4": wo4, "bvrep": bvrep, "misc": misc,
        })

    from concourse.bass_utils import run_bass_kernel_spmd
    trace = bool(os.environ.get("KTRACE"))
    LAST_RESULTS = run_bass_kernel_spmd(nc, in_maps, list(range(N_CORES)),
                                        trace=trace)

    out = np.empty((x.shape[0], DIM), np.float32)
    for c in range(N_CORES):
        oT = LAST_RESULTS.results[c]["outT"]
        for s in range(GPG):
            g = c * GPG + s
            n = int(counts[g])
            out[starts[g]:starts[g] + n] = oT[:, s * GPAD:s * GPAD + n].T
    return out



# revision 67
# speedup vs baseline: 1.3141x; 1.1230x over previous
"""GraphTransformerLayer kernel for 8 Trainium2 NeuronCores.

Sharding: 4 graphs per core (batch is sorted, graphs are contiguous).
Each core runs the full layer (QKV proj -> block-diag attention -> out proj)
on its own graphs; cores are fully independent (no collectives).

Device layout is transposed [feature, node] so every matmul maps onto the
PE array without transposes:
  - qT/kT = W @ xT                       [512, nodes]  (per-feature bias via ACT)
  - v' = x @ Wv'  node-major             [nodes, 520]  (8 heads x (64 dims + ones col))
  - sT[j,i] = k_h^T q_h  per (graph, head); pad-key mask + 1/sqrt(d) scale fused
    into ACT Exp via per-partition bias
  - attn@v with denominator appended as column 65 of v' (ones col via bias)
  - normalize: DVE reciprocal of denom row, K=1 matmul broadcast, DVE multiply
  - o-proj per graph, overlapped with later graphs' attention
All matmul inputs are bf16 (full PE rate), fp32 PSUM accumulate; softmax,
normalization, biases, and the final output stay fp32.
"""

import os
import sys

import numpy as np
import ml_dtypes

for _p in ("/opt/trn_rl_repo", "/root/.axon_site/_ro/trn_rl_repo"):
    if os.path.isdir(_p) and _p not in sys.path:
        sys.path.insert(0, _p)

DIM = 512
H = 8
DH = 64
NUM_GRAPHS = 32
N_CORES = 8
GPG = NUM_GRAPHS // N_CORES  # graphs per core
VC = H * (DH + 1)  # 520: v' columns (per head: 64 dims + 1 ones col)
SCALE = 1.0 / np.sqrt(DH)
NEG = -1e30

_NC_CACHE = {}
LAST_RESULTS = None
LAST_NC = None


def _build(sizes):
    import concourse.bass as bass
    import concourse.tile as tile
    from concourse import mybir
    from contextlib import ExitStack

    f32 = mybir.dt.float32
    b16 = mybir.dt.bfloat16
    AF = mybir.ActivationFunctionType

    # per-slot widths: slot k on every core holds one of the 8 graphs ranked
    # [8k, 8k+8) by size, so S[k] = ceil8(max size in slot k) and padding is
    # minimal; masked j-rows handle slot-boundary overflow exactly as before
    S = list(sizes)
    JT = [-(-s // 128) for s in S]  # j-tiles per slot
    OFF = [0]
    for s in S:
        OFF.append(OFF[-1] + s)
    NPAD = OFF[-1]
    NPX = max(NPAD, OFF[GPG - 1] + 128 * JT[GPG - 1])
    NJT = sum(JT)
    MB = [12 + sum(JT[:k]) for k in range(GPG)]  # mask col base per slot

    def ntiles(total, cap=512):
        out, off = [], 0
        while off < total:
            w = min(cap, total - off)
            out.append((off, w))
            off += w
        return out

    # weight blob (wq|wk|wv|wo): 4 DMAs instead of 16; per-DMA issue costs
    # ~1.2us serialized on HWDGE + the SP sequencer, so fewer, bigger DMAs
    # shorten the prologue
    WB = 2 * DIM + VC  # 1544: wq|wk|wv; wo ships separately (not needed
    # until the first o-proj, so it must not gate the prologue)
    MC = 12 + NJT  # misc fp32 columns: bq|bk|bo|mask
    nc = bass.Bass()
    xT_d = nc.declare_dram_parameter("xT", [DIM, NPX], b16, isOutput=False)
    wb_d = nc.declare_dram_parameter("wb", [DIM, WB], b16, isOutput=False)
    wo4_d = nc.declare_dram_parameter("wo4", [128, 4 * DIM], b16, isOutput=False)
    bv_d = nc.declare_dram_parameter("bvrep", [128, VC], b16, isOutput=False)
    misc_d = nc.declare_dram_parameter("misc", [128, MC], f32, isOutput=False)
    out_d = nc.declare_dram_parameter("outT", [DIM, NPAD], b16, isOutput=True)

    with ExitStack() as ctx:
        tc = ctx.enter_context(tile.TileContext(nc))
        wpool = ctx.enter_context(tc.tile_pool(name="w", bufs=1))
        xpool = ctx.enter_context(tc.tile_pool(name="x", bufs=1))
        apool = ctx.enter_context(tc.tile_pool(name="acts", bufs=1))
        vpool = ctx.enter_context(tc.tile_pool(name="v", bufs=1))
        epool = ctx.enter_context(tc.tile_pool(name="e", bufs=9))
        mpool = ctx.enter_context(tc.tile_pool(name="m", bufs=6))
        opool = ctx.enter_context(tc.tile_pool(name="o", bufs=4))
        ps = ctx.enter_context(tc.tile_pool(name="ps", bufs=4, space="PSUM"))
        avps = ctx.enter_context(tc.tile_pool(name="avps", bufs=2, space="PSUM"))
        bcps = ctx.enter_context(tc.tile_pool(name="bcps", bufs=2, space="PSUM"))

        # interleave weight-blob and x kb-blocks; tiny misc (biases+mask)
        # right after the first pair so early ACT movers never stall on it
        wb_sb, x_sb = [], []
        misc_sb = bv_sb = None
        for kb in range(4):
            w = wpool.tile([128, WB], b16, tag=f"wb{kb}", name=f"wb{kb}")
            nc.sync.dma_start(w[:], wb_d[kb * 128:(kb + 1) * 128, :])
            wb_sb.append(w)
            t = xpool.tile([128, NPX], b16, tag=f"x{kb}", name=f"x{kb}")
            nc.sync.dma_start(t[:], xT_d[kb * 128:(kb + 1) * 128, :])
            x_sb.append(t)
            if kb == 0:
                misc_sb = wpool.tile([128, MC], f32, tag="misc")
                nc.sync.dma_start(misc_sb[:], misc_d[:])
                bv_sb = wpool.tile([128, VC], b16, tag="bv")
                nc.sync.dma_start(bv_sb[:], bv_d[:])
        wo4_sb = wpool.tile([128, 4 * DIM], b16, tag="wo4")
        nc.sync.dma_start(wo4_sb[:], wo4_d[:])
        wq_sb = [w[:, 0:DIM] for w in wb_sb]
        wk_sb = [w[:, DIM:2 * DIM] for w in wb_sb]
        wv_sb = [w[:, 2 * DIM:2 * DIM + VC] for w in wb_sb]
        wo_sb = [wo4_sb[:, kb * DIM:(kb + 1) * DIM] for kb in range(4)]
        bq_sb = misc_sb[:, 0:4]
        bk_sb = misc_sb[:, 4:8]
        bo_sb = misc_sb[:, 8:12]
        mask_sb = misc_sb[:, 12:MC]
        ones_sb = wpool.tile([1, DH], b16, tag="ones")
        nc.gpsimd.memset(ones_sb[:], 1.0)

        # persistent activations
        qT_sb = [apool.tile([128, NPAD], b16, tag=f"q{fb}", name=f"qT{fb}") for fb in range(4)]
        kT_sb = [apool.tile([128, NPX], b16, tag=f"k{fb}", name=f"kT{fb}") for fb in range(4)]
        cT_sb = [apool.tile([128, NPAD], b16, tag=f"c{fb}", name=f"cT{fb}") for fb in range(4)]

        # ---- q / k projections: out[fb*128+m, n] = sum_k W[m,k] x[n,k] + b[m]
        for (w_sb, b_sb, dst, width) in (
            (wq_sb, bq_sb, qT_sb, NPAD),
            (wk_sb, bk_sb, kT_sb, NPX),
        ):
            for fb in range(4):
                for (off, w) in ntiles(width):
                    p = ps.tile([128, 512], f32, tag="ps", name="psb")
                    for kb in range(4):
                        nc.tensor.matmul(
                            p[:, :w],
                            w_sb[kb][:, fb * 128:(fb + 1) * 128],
                            x_sb[kb][:, off:off + w],
                            start=(kb == 0),
                            stop=(kb == 3),
                        )
                    nc.scalar.activation(
                        dst[fb][:, off:off + w], p[:, :w], AF.Identity,
                        bias=b_sb[:, fb:fb + 1],
                    )

        # ---- v' projection (node-major): v[(g,jb)][j, c] for 128-row j tiles
        v_sb = {}
        for g in range(GPG):
            for jb in range(JT[g]):
                vt = vpool.tile([128, VC], b16, tag=f"v{g}_{jb}", name=f"v{g}_{jb}")
                col0 = OFF[g] + jb * 128
                for (off, w) in ntiles(VC):
                    p = ps.tile([128, 512], f32, tag="ps", name="psb")
                    for kb in range(4):
                        nc.tensor.matmul(
                            p[:, :w],
                            x_sb[kb][:, col0:col0 + 128],
                            wv_sb[kb][:, off:off + w],
                            start=(kb == 0),
                            stop=(kb == 3),
                        )
                    nc.vector.tensor_add(vt[:, off:off + w], p[:, :w],
                                         bv_sb[:, off:off + w])
                v_sb[(g, jb)] = vt

        # ---- attention, software-pipelined by (graph, head) step
        def emit_scores(g, h):
            fb, po = h // 2, 64 * (h % 2)
            s = S[g]
            etiles = []
            for jb in range(JT[g]):
                jcol = OFF[g] + jb * 128
                sp = ps.tile([128, 512], f32, tag="ps", name="sps")
                nc.tensor.matmul(
                    sp[:, :s],
                    kT_sb[fb][po:po + 64, jcol:jcol + 128],
                    qT_sb[fb][po:po + 64, OFF[g]:OFF[g] + s],
                    start=True, stop=True,
                    tile_position=(po, 0),
                )
                et = epool.tile([128, 512], b16, tag="e", name="et")
                nc.scalar.activation(
                    et[:, :s], sp[:, :s], AF.Exp,
                    bias=mask_sb[:, MB[g] + jb - 12:MB[g] + jb - 11],
                    scale=float(SCALE),
                )
                etiles.append(et)
            return etiles

        def emit_attnv(g, h, etiles):
            s = S[g]
            op = avps.tile([DH + 1, 512], f32, tag="av", name="avp")
            for jb in range(JT[g]):
                nc.tensor.matmul(
                    op[:, :s],
                    v_sb[(g, jb)][:, 65 * h:65 * h + 65],
                    etiles[jb][:, :s],
                    start=(jb == 0), stop=(jb == JT[g] - 1),
                )
            rc16 = mpool.tile([1, 512], b16, tag="rc16", name="rc16")
            with nc.allow_low_precision(reason="bf16 1/denom: same rounding "
                                        "as the old recip->bf16-copy pair"):
                nc.vector.reciprocal(rc16[:, :s], op[DH:DH + 1, :s])
            return op, rc16

        def emit_norm(g, h, op, rc):
            s = S[g]
            bp = bcps.tile([DH, 512], f32, tag="bc", name="bcp")
            nc.tensor.matmul(bp[:, :s], ones_sb[:], rc[:, :s],
                             start=True, stop=True)
            rb = mpool.tile([DH, 512], f32, tag="rb", name="rb")
            nc.vector.tensor_copy(rb[:, :s], bp[:, :s])
            fb, po = h // 2, 64 * (h % 2)
            nc.vector.tensor_mul(
                cT_sb[fb][po:po + DH, OFF[g]:OFF[g] + s],
                op[0:DH, :s], rb[:, :s],
            )

        def emit_oproj(g):
            s = S[g]
            for fb in range(4):
                p = ps.tile([128, 512], f32, tag="ps", name="psb")
                for kb in range(4):
                    nc.tensor.matmul(
                        p[:, :s],
                        wo_sb[kb][:, fb * 128:(fb + 1) * 128],
                        cT_sb[kb][:, OFF[g]:OFF[g] + s],
                        start=(kb == 0), stop=(kb == 3),
                    )
                ot = opool.tile([128, 512], b16, tag="ot", name="ot")
                nc.scalar.activation(ot[:, :s], p[:, :s], AF.Identity,
                                     bias=bo_sb[:, fb:fb + 1])
                nc.sync.dma_start(
                    out_d[fb * 128:(fb + 1) * 128, OFF[g]:OFF[g] + s],
                    ot[:, :s])

        steps = [(g, h) for g in range(GPG) for h in range(H)]
        NS = len(steps)
        pend = {}
        for t in range(NS + 3):
            if t < NS:
                g, h = steps[t]
                pend[t] = [emit_scores(g, h)]
            if 0 <= t - 2 < NS:
                g, h = steps[t - 2]
                op, rc = emit_attnv(g, h, pend[t - 2][0])
                pend[t - 2] += [op, rc]
            if 0 <= t - 3 < NS:
                g, h = steps[t - 3]
                _, op, rc = pend.pop(t - 3)
                emit_norm(g, h, op, rc)
                if h == H - 1:
                    emit_oproj(g)

    _split_multiwaits(nc, mybir)
    return nc, dict(S=S, JT=JT, OFF=OFF, NPAD=NPAD, NPX=NPX)


def _split_multiwaits(nc, mybir, max_waits=1):
    """The pinned walrus codegen accepts only one sync-wait per instruction;
    move extra waits onto dedicated NoOps just before the instruction (same
    engine stream, so semantics are identical)."""
    n_split = 0
    for fn in nc.m.functions:
        for blk in fn.blocks:
            new_insts = []
            for inst in blk.instructions:
                si = getattr(inst, "sync_info", None)
                if si is not None and si.on_wait and len(si.on_wait) > max_waits:
                    waits = list(si.on_wait)
                    extra, keep = waits[:-max_waits], waits[-max_waits:]
                    for i, w in enumerate(extra):
                        new_insts.append(mybir.InstNoOp(
                            name=f"{inst.name}-w{i}",
                            sync_info=mybir.SyncInfo(on_wait=[w], on_update=[]),
                            engine=inst.engine,
                            bass_nofuse=True,
                        ))
                    inst.sync_info = mybir.SyncInfo(on_wait=keep,
                                                    on_update=si.on_update)
                    n_split += 1
                new_insts.append(inst)
            blk.instructions = new_insts
    return n_split


def _get_nc(sizes):
    sizes = tuple(sizes)
    if sizes not in _NC_CACHE:
        _NC_CACHE[sizes] = _build(sizes)
    return _NC_CACHE[sizes]


def kernel(x, batch, w_q, w_k, w_v, b_q, b_k, b_v, w_o, b_o):
    global LAST_RESULTS, LAST_NC
    x = np.asarray(x, np.float32)
    batch = np.asarray(batch, np.int64)
    counts = np.bincount(batch, minlength=NUM_GRAPHS)[:NUM_GRAPHS]
    starts = np.concatenate([[0], np.cumsum(counts)]).astype(np.int64)
    # slot k on every core gets one of the 8 graphs ranked [8k, 8k+8) by
    # size, so the shared slot width is the k-th octile max (ceil8)
    order = np.argsort(-counts, kind="stable")
    sizes = tuple(int(-(-int(counts[order[8 * k]]) // 8) * 8)
                  for k in range(GPG))
    assert all(s <= 512 for s in sizes), f"graph too large: {counts.max()}"
    nc, meta = _get_nc(sizes)
    LAST_NC = nc
    JT, OFF, NPAD, NPX = meta["JT"], meta["OFF"], meta["NPAD"], meta["NPX"]
    MB = [sum(JT[:k]) for k in range(GPG)]

    # shared host-side prepped weights (bf16 for matmul operands)
    bf16 = ml_dtypes.bfloat16
    wvT = np.zeros((DIM, VC), np.float32)
    bvp = np.zeros(VC, np.float32)
    for h in range(H):
        wvT[:, 65 * h:65 * h + 64] = w_v[64 * h:64 * h + 64, :].T
        bvp[65 * h:65 * h + 64] = b_v[64 * h:64 * h + 64]
        bvp[65 * h + 64] = 1.0
    wb = np.concatenate([w_q.T, w_k.T, wvT], axis=1)
    wb = np.ascontiguousarray(wb).astype(bf16)
    woT = w_o.T
    wo4 = np.concatenate([woT[kb * 128:(kb + 1) * 128] for kb in range(4)],
                         axis=1)
    wo4 = np.ascontiguousarray(wo4).astype(bf16)
    bvrep = np.ascontiguousarray(np.broadcast_to(bvp, (128, VC))).astype(bf16)
    misc0 = np.empty((128, 12 + sum(JT)), np.float32)
    misc0[:, 0:4] = b_q.reshape(4, 128).T
    misc0[:, 4:8] = b_k.reshape(4, 128).T
    misc0[:, 8:12] = b_o.reshape(4, 128).T

    in_maps = []
    for c in range(N_CORES):
        xs = np.zeros((NPX, DIM), np.float32)
        misc = misc0.copy()
        misc[:, 12:] = NEG
        for k in range(GPG):
            g = order[8 * k + c]
            n = int(counts[g])
            xs[OFF[k]:OFF[k] + n] = x[starts[g]:starts[g] + n]
            for jb in range(JT[k]):
                valid = np.arange(128) + jb * 128 < n
                misc[valid, 12 + MB[k] + jb] = 0.0
        in_maps.append({
            "xT": np.ascontiguousarray(xs.T).astype(bf16),
            "wb": wb, "wo4": wo4, "bvrep": bvrep, "misc": misc,
        })

    from concourse.bass_utils import run_bass_kernel_spmd
    trace = bool(os.environ.get("KTRACE"))
    LAST_RESULTS = run_bass_kernel_spmd(nc, in_maps, list(range(N_CORES)),
                                        trace=trace)

    out = np.empty((x.shape[0], DIM), np.float32)
    for c in range(N_CORES):
        oT = LAST_RESULTS.results[c]["outT"]
        for k in range(GPG):
            g = order[8 * k + c]
            n = int(counts[g])
            out[starts[g]:starts[g] + n] = oT[:, OFF[k]:OFF[k] + n].T
    return out



# revision 77
# speedup vs baseline: 1.3486x; 1.0263x over previous
"""GraphTransformerLayer kernel for 8 Trainium2 NeuronCores.

Sharding: 4 graphs per core (batch is sorted, graphs are contiguous).
Each core runs the full layer (QKV proj -> block-diag attention -> out proj)
on its own graphs; cores are fully independent (no collectives).

Device layout is transposed [feature, node] so every matmul maps onto the
PE array without transposes:
  - qT/kT = W @ xT                       [512, nodes]  (per-feature bias via ACT)
  - v' = x @ Wv'  node-major             [nodes, 520]  (8 heads x (64 dims + ones col))
  - sT[j,i] = k_h^T q_h  per (graph, head); pad-key mask + 1/sqrt(d) scale fused
    into ACT Exp via per-partition bias
  - attn@v with denominator appended as column 65 of v' (ones col via bias)
  - normalize: DVE reciprocal of denom row, K=1 matmul broadcast, DVE multiply
  - o-proj per graph, overlapped with later graphs' attention
All matmul inputs are bf16 (full PE rate), fp32 PSUM accumulate; softmax,
normalization, biases, and the final output stay fp32.
"""

import os
import sys

import numpy as np
import ml_dtypes

for _p in ("/opt/trn_rl_repo", "/root/.axon_site/_ro/trn_rl_repo"):
    if os.path.isdir(_p) and _p not in sys.path:
        sys.path.insert(0, _p)

DIM = 512
H = 8
DH = 64
NUM_GRAPHS = 32
N_CORES = 8
GPG = NUM_GRAPHS // N_CORES  # graphs per core
VC = H * (DH + 1)  # 520: v' columns (per head: 64 dims + 1 ones col)
SCALE = 1.0 / np.sqrt(DH)
NEG = -1e30

_NC_CACHE = {}
LAST_RESULTS = None
LAST_NC = None


def _build(sizes):
    import concourse.bass as bass
    import concourse.tile as tile
    from concourse import mybir
    from contextlib import ExitStack

    f32 = mybir.dt.float32
    b16 = mybir.dt.bfloat16
    AF = mybir.ActivationFunctionType

    # per-slot widths: slot k on every core holds one of the 8 graphs ranked
    # [8k, 8k+8) by size, so S[k] = ceil8(max size in slot k) and padding is
    # minimal; masked j-rows handle slot-boundary overflow exactly as before
    S = list(sizes)
    JT = [-(-s // 128) for s in S]  # j-tiles per slot
    OFF = [0]
    for s in S:
        OFF.append(OFF[-1] + s)
    NPAD = OFF[-1]
    NPX = max(NPAD, OFF[GPG - 1] + 128 * JT[GPG - 1])
    NJT = sum(JT)
    MB = [12 + sum(JT[:k]) for k in range(GPG)]  # mask col base per slot

    def ntiles(total, cap=512):
        out, off = [], 0
        while off < total:
            w = min(cap, total - off)
            out.append((off, w))
            off += w
        return out

    # weight blob (wq|wk|wv|wo): 4 DMAs instead of 16; per-DMA issue costs
    # ~1.2us serialized on HWDGE + the SP sequencer, so fewer, bigger DMAs
    # shorten the prologue
    WB = 2 * DIM + VC  # 1544: wq|wk|wv; wo ships separately (not needed
    # until the first o-proj, so it must not gate the prologue)
    MC = 12 + NJT  # misc fp32 columns: bq|bk|bo|mask
    nc = bass.Bass()
    xT_d = nc.declare_dram_parameter("xT", [DIM, NPX], b16, isOutput=False)
    wb_d = nc.declare_dram_parameter("wb", [DIM, WB], b16, isOutput=False)
    wo4_d = nc.declare_dram_parameter("wo4", [128, 4 * DIM], b16, isOutput=False)
    bv_d = nc.declare_dram_parameter("bvrep", [128, VC], b16, isOutput=False)
    misc_d = nc.declare_dram_parameter("misc", [128, MC], f32, isOutput=False)
    out_d = nc.declare_dram_parameter("outT", [DIM, NPAD], b16, isOutput=True)

    with ExitStack() as ctx:
        tc = ctx.enter_context(tile.TileContext(nc))
        wpool = ctx.enter_context(tc.tile_pool(name="w", bufs=1))
        xpool = ctx.enter_context(tc.tile_pool(name="x", bufs=1))
        apool = ctx.enter_context(tc.tile_pool(name="acts", bufs=1))
        vpool = ctx.enter_context(tc.tile_pool(name="v", bufs=1))
        epool = ctx.enter_context(tc.tile_pool(name="e", bufs=12))
        mpool = ctx.enter_context(tc.tile_pool(name="m", bufs=8))
        opool = ctx.enter_context(tc.tile_pool(name="o", bufs=6))
        ps = ctx.enter_context(tc.tile_pool(name="ps", bufs=4, space="PSUM"))
        avps = ctx.enter_context(tc.tile_pool(name="avps", bufs=2, space="PSUM"))
        bcps = ctx.enter_context(tc.tile_pool(name="bcps", bufs=2, space="PSUM"))

        # interleave weight-blob and x kb-blocks; tiny misc (biases+mask)
        # right after the first pair so early ACT movers never stall on it
        wb_sb, x_sb = [], []
        misc_sb = bv_sb = None
        for kb in range(4):
            w = wpool.tile([128, WB], b16, tag=f"wb{kb}", name=f"wb{kb}")
            nc.sync.dma_start(w[:], wb_d[kb * 128:(kb + 1) * 128, :])
            wb_sb.append(w)
            t = xpool.tile([128, NPX], b16, tag=f"x{kb}", name=f"x{kb}")
            nc.sync.dma_start(t[:], xT_d[kb * 128:(kb + 1) * 128, :])
            x_sb.append(t)
            if kb == 0:
                misc_sb = wpool.tile([128, MC], f32, tag="misc")
                nc.sync.dma_start(misc_sb[:], misc_d[:])
                bv_sb = wpool.tile([128, VC], b16, tag="bv")
                nc.sync.dma_start(bv_sb[:], bv_d[:])
        wo4_sb = wpool.tile([128, 4 * DIM], b16, tag="wo4")
        nc.sync.dma_start(wo4_sb[:], wo4_d[:])
        wq_sb = [w[:, 0:DIM] for w in wb_sb]
        wk_sb = [w[:, DIM:2 * DIM] for w in wb_sb]
        wv_sb = [w[:, 2 * DIM:2 * DIM + VC] for w in wb_sb]
        wo_sb = [wo4_sb[:, kb * DIM:(kb + 1) * DIM] for kb in range(4)]
        bq_sb = misc_sb[:, 0:4]
        bk_sb = misc_sb[:, 4:8]
        bo_sb = misc_sb[:, 8:12]
        mask_sb = misc_sb[:, 12:MC]
        ones_sb = wpool.tile([1, DH], b16, tag="ones")
        nc.gpsimd.memset(ones_sb[:], 1.0)

        # persistent activations
        qT_sb = [apool.tile([128, NPAD], b16, tag=f"q{fb}", name=f"qT{fb}") for fb in range(4)]
        kT_sb = [apool.tile([128, NPX], b16, tag=f"k{fb}", name=f"kT{fb}") for fb in range(4)]
        cT_sb = [apool.tile([128, NPAD], b16, tag=f"c{fb}", name=f"cT{fb}") for fb in range(4)]

        # ---- q / k projections: out[fb*128+m, n] = sum_k W[m,k] x[n,k] + b[m]
        for (w_sb, b_sb, dst, width) in (
            (wq_sb, bq_sb, qT_sb, NPAD),
            (wk_sb, bk_sb, kT_sb, NPX),
        ):
            for fb in range(4):
                for (off, w) in ntiles(width):
                    p = ps.tile([128, 512], f32, tag="ps", name="psb")
                    for kb in range(4):
                        nc.tensor.matmul(
                            p[:, :w],
                            w_sb[kb][:, fb * 128:(fb + 1) * 128],
                            x_sb[kb][:, off:off + w],
                            start=(kb == 0),
                            stop=(kb == 3),
                        )
                    nc.scalar.activation(
                        dst[fb][:, off:off + w], p[:, :w], AF.Identity,
                        bias=b_sb[:, fb:fb + 1],
                    )

        # ---- v' projection (node-major): v[(g,jb)][j, c] for 128-row j
        # tiles. Only slot 0's tiles are built up front; later slots'
        # tiles are emitted inside the previous slot's attention steps so
        # the PE-only v phase overlaps ACT/DVE-heavy attention work.
        v_sb = {}

        def emit_v_unit(g, jb):
            vt = vpool.tile([128, VC], b16, tag=f"v{g}_{jb}", name=f"v{g}_{jb}")
            col0 = OFF[g] + jb * 128
            for (off, w) in ntiles(VC):
                p = ps.tile([128, 512], f32, tag="ps", name="psb")
                for kb in range(4):
                    nc.tensor.matmul(
                        p[:, :w],
                        x_sb[kb][:, col0:col0 + 128],
                        wv_sb[kb][:, off:off + w],
                        start=(kb == 0),
                        stop=(kb == 3),
                    )
                nc.vector.tensor_add(vt[:, off:off + w], p[:, :w],
                                     bv_sb[:, off:off + w])
            v_sb[(g, jb)] = vt

        for jb in range(JT[0]):
            emit_v_unit(0, jb)

        # ---- attention, software-pipelined by (graph, head) step
        def emit_scores(g, h):
            fb, po = h // 2, 64 * (h % 2)
            s = S[g]
            etiles = []
            for jb in range(JT[g]):
                jcol = OFF[g] + jb * 128
                sp = ps.tile([128, 512], f32, tag="ps", name="sps")
                nc.tensor.matmul(
                    sp[:, :s],
                    kT_sb[fb][po:po + 64, jcol:jcol + 128],
                    qT_sb[fb][po:po + 64, OFF[g]:OFF[g] + s],
                    start=True, stop=True,
                    tile_position=(po, 0),
                )
                et = epool.tile([128, 512], b16, tag="e", name="et")
                nc.scalar.activation(
                    et[:, :s], sp[:, :s], AF.Exp,
                    bias=mask_sb[:, MB[g] + jb - 12:MB[g] + jb - 11],
                    scale=float(SCALE),
                )
                etiles.append(et)
            return etiles

        def emit_attnv(g, h, etiles):
            s = S[g]
            op = avps.tile([DH + 1, 512], f32, tag="av", name="avp")
            for jb in range(JT[g]):
                nc.tensor.matmul(
                    op[:, :s],
                    v_sb[(g, jb)][:, 65 * h:65 * h + 65],
                    etiles[jb][:, :s],
                    start=(jb == 0), stop=(jb == JT[g] - 1),
                )
            rc16 = mpool.tile([1, 512], b16, tag="rc16", name="rc16")
            with nc.allow_low_precision(reason="bf16 1/denom: same rounding "
                                        "as the old recip->bf16-copy pair"):
                nc.vector.reciprocal(rc16[:, :s], op[DH:DH + 1, :s])
            return op, rc16

        def emit_norm(g, h, op, rc):
            s = S[g]
            bp = bcps.tile([DH, 512], f32, tag="bc", name="bcp")
            nc.tensor.matmul(bp[:, :s], ones_sb[:], rc[:, :s],
                             start=True, stop=True)
            rb = mpool.tile([DH, 512], f32, tag="rb", name="rb")
            nc.vector.tensor_copy(rb[:, :s], bp[:, :s])
            fb, po = h // 2, 64 * (h % 2)
            nc.vector.tensor_mul(
                cT_sb[fb][po:po + DH, OFF[g]:OFF[g] + s],
                op[0:DH, :s], rb[:, :s],
            )

        def emit_oproj(g):
            s = S[g]
            for fb in range(4):
                p = ps.tile([128, 512], f32, tag="ps", name="psb")
                for kb in range(4):
                    nc.tensor.matmul(
                        p[:, :s],
                        wo_sb[kb][:, fb * 128:(fb + 1) * 128],
                        cT_sb[kb][:, OFF[g]:OFF[g] + s],
                        start=(kb == 0), stop=(kb == 3),
                    )
                ot = opool.tile([128, 512], b16, tag="ot", name="ot")
                nc.scalar.activation(ot[:, :s], p[:, :s], AF.Identity,
                                     bias=bo_sb[:, fb:fb + 1])
                nc.sync.dma_start(
                    out_d[fb * 128:(fb + 1) * 128, OFF[g]:OFF[g] + s],
                    ot[:, :s])

        steps = [(g, h) for g in range(GPG) for h in range(H)]
        NS = len(steps)
        pend = {}
        for t in range(NS + 3):
            if t < NS:
                g, h = steps[t]
                pend[t] = [emit_scores(g, h)]
                # one v-unit of the NEXT slot at h = 0/3/6 (<=3 j-tiles)
                if g + 1 < GPG and h % 3 == 0 and h // 3 < JT[g + 1]:
                    emit_v_unit(g + 1, h // 3)
            if 0 <= t - 2 < NS:
                g, h = steps[t - 2]
                op, rc = emit_attnv(g, h, pend[t - 2][0])
                pend[t - 2] += [op, rc]
            if 0 <= t - 3 < NS:
                g, h = steps[t - 3]
                _, op, rc = pend.pop(t - 3)
                emit_norm(g, h, op, rc)
                if h == H - 1:
                    emit_oproj(g)

    _split_multiwaits(nc, mybir)
    return nc, dict(S=S, JT=JT, OFF=OFF, NPAD=NPAD, NPX=NPX)


def _split_multiwaits(nc, mybir, max_waits=1):
    """The pinned walrus codegen accepts only one sync-wait per instruction;
    move extra waits onto dedicated NoOps just before the instruction (same
    engine stream, so semantics are identical)."""
    n_split = 0
    for fn in nc.m.functions:
        for blk in fn.blocks:
            new_insts = []
            for inst in blk.instructions:
                si = getattr(inst, "sync_info", None)
                if si is not None and si.on_wait and len(si.on_wait) > max_waits:
                    waits = list(si.on_wait)
                    extra, keep = waits[:-max_waits], waits[-max_waits:]
                    for i, w in enumerate(extra):
                        new_insts.append(mybir.InstNoOp(
                            name=f"{inst.name}-w{i}",
                            sync_info=mybir.SyncInfo(on_wait=[w], on_update=[]),
                            engine=inst.engine,
                            bass_nofuse=True,
                        ))
                    inst.sync_info = mybir.SyncInfo(on_wait=keep,
                                                    on_update=si.on_update)
                    n_split += 1
                new_insts.append(inst)
            blk.instructions = new_insts
    return n_split


def _get_nc(sizes):
    sizes = tuple(sizes)
    if sizes not in _NC_CACHE:
        _NC_CACHE[sizes] = _build(sizes)
    return _NC_CACHE[sizes]


def kernel(x, batch, w_q, w_k, w_v, b_q, b_k, b_v, w_o, b_o):
    global LAST_RESULTS, LAST_NC
    x = np.asarray(x, np.float32)
    batch = np.asarray(batch, np.int64)
    counts = np.bincount(batch, minlength=NUM_GRAPHS)[:NUM_GRAPHS]
    starts = np.concatenate([[0], np.cumsum(counts)]).astype(np.int64)
    # slot k on every core gets one of the 8 graphs ranked [8k, 8k+8) by
    # size, so the shared slot width is the k-th octile max (ceil8)
    order = np.argsort(-counts, kind="stable")
    sizes = tuple(int(-(-int(counts[order[8 * k]]) // 8) * 8)
                  for k in range(GPG))
    assert all(s <= 512 for s in sizes), f"graph too large: {counts.max()}"
    nc, meta = _get_nc(sizes)
    LAST_NC = nc
    JT, OFF, NPAD, NPX = meta["JT"], meta["OFF"], meta["NPAD"], meta["NPX"]
    MB = [sum(JT[:k]) for k in range(GPG)]

    # shared host-side prepped weights (bf16 for matmul operands)
    bf16 = ml_dtypes.bfloat16
    wvT = np.zeros((DIM, VC), np.float32)
    bvp = np.zeros(VC, np.float32)
    for h in range(H):
        wvT[:, 65 * h:65 * h + 64] = w_v[64 * h:64 * h + 64, :].T
        bvp[65 * h:65 * h + 64] = b_v[64 * h:64 * h + 64]
        bvp[65 * h + 64] = 1.0
    wb = np.concatenate([w_q.T, w_k.T, wvT], axis=1)
    wb = np.ascontiguousarray(wb).astype(bf16)
    woT = w_o.T
    wo4 = np.concatenate([woT[kb * 128:(kb + 1) * 128] for kb in range(4)],
                         axis=1)
    wo4 = np.ascontiguousarray(wo4).astype(bf16)
    bvrep = np.ascontiguousarray(np.broadcast_to(bvp, (128, VC))).astype(bf16)
    misc0 = np.empty((128, 12 + sum(JT)), np.float32)
    misc0[:, 0:4] = b_q.reshape(4, 128).T
    misc0[:, 4:8] = b_k.reshape(4, 128).T
    misc0[:, 8:12] = b_o.reshape(4, 128).T

    in_maps = []
    for c in range(N_CORES):
        xs = np.zeros((NPX, DIM), np.float32)
        misc = misc0.copy()
        misc[:, 12:] = NEG
        for k in range(GPG):
            g = order[8 * k + c]
            n = int(counts[g])
            xs[OFF[k]:OFF[k] + n] = x[starts[g]:starts[g] + n]
            for jb in range(JT[k]):
                valid = np.arange(128) + jb * 128 < n
                misc[valid, 12 + MB[k] + jb] = 0.0
        in_maps.append({
            "xT": np.ascontiguousarray(xs.T).astype(bf16),
            "wb": wb, "wo4": wo4, "bvrep": bvrep, "misc": misc,
        })

    from concourse.bass_utils import run_bass_kernel_spmd
    trace = bool(os.environ.get("KTRACE"))
    LAST_RESULTS = run_bass_kernel_spmd(nc, in_maps, list(range(N_CORES)),
                                        trace=trace)

    out = np.empty((x.shape[0], DIM), np.float32)
    for c in range(N_CORES):
        oT = LAST_RESULTS.results[c]["outT"]
        for k in range(GPG):
            g = order[8 * k + c]
            n = int(counts[g])
            out[starts[g]:starts[g] + n] = oT[:, OFF[k]:OFF[k] + n].T
    return out



# revision 79
# speedup vs baseline: 1.3732x; 1.0183x over previous
"""GraphTransformerLayer kernel for 8 Trainium2 NeuronCores.

Sharding: 4 graphs per core (batch is sorted, graphs are contiguous).
Each core runs the full layer (QKV proj -> block-diag attention -> out proj)
on its own graphs; cores are fully independent (no collectives).

Device layout is transposed [feature, node] so every matmul maps onto the
PE array without transposes:
  - qT/kT = W @ xT                       [512, nodes]  (per-feature bias via ACT)
  - v' = x @ Wv'  node-major             [nodes, 520]  (8 heads x (64 dims + ones col))
  - sT[j,i] = k_h^T q_h  per (graph, head); pad-key mask + 1/sqrt(d) scale fused
    into ACT Exp via per-partition bias
  - attn@v with denominator appended as column 65 of v' (ones col via bias)
  - normalize: DVE reciprocal of denom row, K=1 matmul broadcast, DVE multiply
  - o-proj per graph, overlapped with later graphs' attention
All matmul inputs are bf16 (full PE rate), fp32 PSUM accumulate; softmax,
normalization, biases, and the final output stay fp32.
"""

import os
import sys

import numpy as np
import ml_dtypes

for _p in ("/opt/trn_rl_repo", "/root/.axon_site/_ro/trn_rl_repo"):
    if os.path.isdir(_p) and _p not in sys.path:
        sys.path.insert(0, _p)

DIM = 512
H = 8
DH = 64
NUM_GRAPHS = 32
N_CORES = 8
GPG = NUM_GRAPHS // N_CORES  # graphs per core
VC = H * (DH + 1)  # 520: v' columns (per head: 64 dims + 1 ones col)
SCALE = 1.0 / np.sqrt(DH)
NEG = -1e30

_NC_CACHE = {}
LAST_RESULTS = None
LAST_NC = None


def _build(sizes):
    import concourse.bass as bass
    import concourse.tile as tile
    from concourse import mybir
    from contextlib import ExitStack

    f32 = mybir.dt.float32
    b16 = mybir.dt.bfloat16
    AF = mybir.ActivationFunctionType

    # per-slot widths: slot k on every core holds one of the 8 graphs ranked
    # [8k, 8k+8) by size, so S[k] = ceil8(max size in slot k) and padding is
    # minimal; masked j-rows handle slot-boundary overflow exactly as before
    S = list(sizes)
    JT = [-(-s // 128) for s in S]  # j-tiles per slot
    OFF = [0]
    for s in S:
        OFF.append(OFF[-1] + s)
    NPAD = OFF[-1]
    NPX = max(NPAD, OFF[GPG - 1] + 128 * JT[GPG - 1])
    NJT = sum(JT)
    MB = [12 + sum(JT[:k]) for k in range(GPG)]  # mask col base per slot

    def ntiles(total, cap=512):
        out, off = [], 0
        while off < total:
            w = min(cap, total - off)
            out.append((off, w))
            off += w
        return out

    # weight blob (wq|wk|wv|wo): 4 DMAs instead of 16; per-DMA issue costs
    # ~1.2us serialized on HWDGE + the SP sequencer, so fewer, bigger DMAs
    # shorten the prologue
    WB = 2 * DIM + VC  # 1544: wq|wk|wv; wo ships separately (not needed
    # until the first o-proj, so it must not gate the prologue)
    MC = 12 + NJT  # misc fp32 columns: bq|bk|bo|mask
    nc = bass.Bass()
    xT_d = nc.declare_dram_parameter("xT", [DIM, NPX], b16, isOutput=False)
    wb_d = nc.declare_dram_parameter("wb", [DIM, WB], b16, isOutput=False)
    wo4_d = nc.declare_dram_parameter("wo4", [128, 4 * DIM], b16, isOutput=False)
    bv_d = nc.declare_dram_parameter("bvrep", [128, VC], b16, isOutput=False)
    misc_d = nc.declare_dram_parameter("misc", [128, MC], f32, isOutput=False)
    out_d = nc.declare_dram_parameter("outT", [DIM, NPAD], b16, isOutput=True)

    with ExitStack() as ctx:
        tc = ctx.enter_context(tile.TileContext(nc))
        wpool = ctx.enter_context(tc.tile_pool(name="w", bufs=1))
        xpool = ctx.enter_context(tc.tile_pool(name="x", bufs=1))
        apool = ctx.enter_context(tc.tile_pool(name="acts", bufs=1))
        vpool = ctx.enter_context(tc.tile_pool(name="v", bufs=1))
        epool = ctx.enter_context(tc.tile_pool(name="e", bufs=12))
        mpool = ctx.enter_context(tc.tile_pool(name="m", bufs=8))
        opool = ctx.enter_context(tc.tile_pool(name="o", bufs=6))
        ps = ctx.enter_context(tc.tile_pool(name="ps", bufs=4, space="PSUM"))
        avps = ctx.enter_context(tc.tile_pool(name="avps", bufs=2, space="PSUM"))
        bcps = ctx.enter_context(tc.tile_pool(name="bcps", bufs=2, space="PSUM"))

        # interleave weight-blob and x kb-blocks; tiny misc (biases+mask)
        # right after the first pair so early ACT movers never stall on it
        wb_sb, x_sb = [], []
        misc_sb = bv_sb = None
        for kb in range(4):
            w = wpool.tile([128, WB], b16, tag=f"wb{kb}", name=f"wb{kb}")
            nc.sync.dma_start(w[:], wb_d[kb * 128:(kb + 1) * 128, :])
            wb_sb.append(w)
            t = xpool.tile([128, NPX], b16, tag=f"x{kb}", name=f"x{kb}")
            nc.sync.dma_start(t[:], xT_d[kb * 128:(kb + 1) * 128, :])
            x_sb.append(t)
            if kb == 0:
                misc_sb = wpool.tile([128, MC], f32, tag="misc")
                nc.sync.dma_start(misc_sb[:], misc_d[:])
                bv_sb = wpool.tile([128, VC], b16, tag="bv")
                nc.sync.dma_start(bv_sb[:], bv_d[:])
        wo4_sb = wpool.tile([128, 4 * DIM], b16, tag="wo4")
        nc.sync.dma_start(wo4_sb[:], wo4_d[:])
        wq_sb = [w[:, 0:DIM] for w in wb_sb]
        wk_sb = [w[:, DIM:2 * DIM] for w in wb_sb]
        wv_sb = [w[:, 2 * DIM:2 * DIM + VC] for w in wb_sb]
        wo_sb = [wo4_sb[:, kb * DIM:(kb + 1) * DIM] for kb in range(4)]
        bq_sb = misc_sb[:, 0:4]
        bk_sb = misc_sb[:, 4:8]
        bo_sb = misc_sb[:, 8:12]
        mask_sb = misc_sb[:, 12:MC]
        ones_sb = wpool.tile([1, DH], b16, tag="ones")
        nc.gpsimd.memset(ones_sb[:], 1.0)

        # persistent activations
        qT_sb = [apool.tile([128, NPAD], b16, tag=f"q{fb}", name=f"qT{fb}") for fb in range(4)]
        kT_sb = [apool.tile([128, NPX], b16, tag=f"k{fb}", name=f"kT{fb}") for fb in range(4)]
        cT_sb = [apool.tile([128, NPAD], b16, tag=f"c{fb}", name=f"cT{fb}") for fb in range(4)]

        # ---- q / k projections: out[fb*128+m, n] = sum_k W[m,k] x[n,k] + b[m]
        # k is built fully up front (slot j-spans overflow into the next
        # slot's columns); q is per-slot -- only slot 0 up front, later
        # slots' q-units ride inside the previous slot's attention steps
        def emit_q_slot(k, fb):
            s = S[k]
            p = ps.tile([128, 512], f32, tag="ps", name="psb")
            for kb in range(4):
                nc.tensor.matmul(
                    p[:, :s],
                    wq_sb[kb][:, fb * 128:(fb + 1) * 128],
                    x_sb[kb][:, OFF[k]:OFF[k] + s],
                    start=(kb == 0), stop=(kb == 3),
                )
            nc.scalar.activation(
                qT_sb[fb][:, OFF[k]:OFF[k] + s], p[:, :s], AF.Identity,
                bias=bq_sb[:, fb:fb + 1],
            )

        for fb in range(4):
            emit_q_slot(0, fb)
        for fb in range(4):
            for (off, w) in ntiles(NPX):
                p = ps.tile([128, 512], f32, tag="ps", name="psb")
                for kb in range(4):
                    nc.tensor.matmul(
                        p[:, :w],
                        wk_sb[kb][:, fb * 128:(fb + 1) * 128],
                        x_sb[kb][:, off:off + w],
                        start=(kb == 0),
                        stop=(kb == 3),
                    )
                nc.scalar.activation(
                    kT_sb[fb][:, off:off + w], p[:, :w], AF.Identity,
                    bias=bk_sb[:, fb:fb + 1],
                )

        # ---- v' projection (node-major): v[(g,jb)][j, c] for 128-row j
        # tiles. Only slot 0's tiles are built up front; later slots'
        # tiles are emitted inside the previous slot's attention steps so
        # the PE-only v phase overlaps ACT/DVE-heavy attention work.
        v_sb = {}

        def emit_v_unit(g, jb):
            vt = vpool.tile([128, VC], b16, tag=f"v{g}_{jb}", name=f"v{g}_{jb}")
            col0 = OFF[g] + jb * 128
            for (off, w) in ntiles(VC):
                p = ps.tile([128, 512], f32, tag="ps", name="psb")
                for kb in range(4):
                    nc.tensor.matmul(
                        p[:, :w],
                        x_sb[kb][:, col0:col0 + 128],
                        wv_sb[kb][:, off:off + w],
                        start=(kb == 0),
                        stop=(kb == 3),
                    )
                nc.vector.tensor_add(vt[:, off:off + w], p[:, :w],
                                     bv_sb[:, off:off + w])
            v_sb[(g, jb)] = vt

        for jb in range(JT[0]):
            emit_v_unit(0, jb)

        # ---- attention, software-pipelined by (graph, head) step
        def emit_scores(g, h):
            fb, po = h // 2, 64 * (h % 2)
            s = S[g]
            etiles = []
            for jb in range(JT[g]):
                jcol = OFF[g] + jb * 128
                sp = ps.tile([128, 512], f32, tag="ps", name="sps")
                nc.tensor.matmul(
                    sp[:, :s],
                    kT_sb[fb][po:po + 64, jcol:jcol + 128],
                    qT_sb[fb][po:po + 64, OFF[g]:OFF[g] + s],
                    start=True, stop=True,
                    tile_position=(po, 0),
                )
                et = epool.tile([128, 512], b16, tag="e", name="et")
                nc.scalar.activation(
                    et[:, :s], sp[:, :s], AF.Exp,
                    bias=mask_sb[:, MB[g] + jb - 12:MB[g] + jb - 11],
                    scale=float(SCALE),
                )
                etiles.append(et)
            return etiles

        def emit_attnv(g, h, etiles):
            s = S[g]
            op = avps.tile([DH + 1, 512], f32, tag="av", name="avp")
            for jb in range(JT[g]):
                nc.tensor.matmul(
                    op[:, :s],
                    v_sb[(g, jb)][:, 65 * h:65 * h + 65],
                    etiles[jb][:, :s],
                    start=(jb == 0), stop=(jb == JT[g] - 1),
                )
            rc16 = mpool.tile([1, 512], b16, tag="rc16", name="rc16")
            with nc.allow_low_precision(reason="bf16 1/denom: same rounding "
                                        "as the old recip->bf16-copy pair"):
                nc.vector.reciprocal(rc16[:, :s], op[DH:DH + 1, :s])
            return op, rc16

        def emit_norm(g, h, op, rc):
            s = S[g]
            bp = bcps.tile([DH, 512], f32, tag="bc", name="bcp")
            nc.tensor.matmul(bp[:, :s], ones_sb[:], rc[:, :s],
                             start=True, stop=True)
            rb = mpool.tile([DH, 512], f32, tag="rb", name="rb")
            nc.vector.tensor_copy(rb[:, :s], bp[:, :s])
            fb, po = h // 2, 64 * (h % 2)
            nc.vector.tensor_mul(
                cT_sb[fb][po:po + DH, OFF[g]:OFF[g] + s],
                op[0:DH, :s], rb[:, :s],
            )

        def emit_oproj(g):
            s = S[g]
            for fb in range(4):
                p = ps.tile([128, 512], f32, tag="ps", name="psb")
                for kb in range(4):
                    nc.tensor.matmul(
                        p[:, :s],
                        wo_sb[kb][:, fb * 128:(fb + 1) * 128],
                        cT_sb[kb][:, OFF[g]:OFF[g] + s],
                        start=(kb == 0), stop=(kb == 3),
                    )
                ot = opool.tile([128, 512], b16, tag="ot", name="ot")
                nc.scalar.activation(ot[:, :s], p[:, :s], AF.Identity,
                                     bias=bo_sb[:, fb:fb + 1])
                nc.sync.dma_start(
                    out_d[fb * 128:(fb + 1) * 128, OFF[g]:OFF[g] + s],
                    ot[:, :s])

        steps = [(g, h) for g in range(GPG) for h in range(H)]
        NS = len(steps)
        pend = {}
        for t in range(NS + 3):
            if t < NS:
                g, h = steps[t]
                pend[t] = [emit_scores(g, h)]
                # one v-unit of the NEXT slot at h = 0/3/6 (<=3 j-tiles)
                if g + 1 < GPG and h % 3 == 0 and h // 3 < JT[g + 1]:
                    emit_v_unit(g + 1, h // 3)
                # one q-unit of the NEXT slot at h = 1/3/5/7
                if g + 1 < GPG and h % 2 == 1:
                    emit_q_slot(g + 1, h // 2)
            if 0 <= t - 2 < NS:
                g, h = steps[t - 2]
                op, rc = emit_attnv(g, h, pend[t - 2][0])
                pend[t - 2] += [op, rc]
            if 0 <= t - 3 < NS:
                g, h = steps[t - 3]
                _, op, rc = pend.pop(t - 3)
                emit_norm(g, h, op, rc)
                if h == H - 1:
                    emit_oproj(g)

    _split_multiwaits(nc, mybir)
    return nc, dict(S=S, JT=JT, OFF=OFF, NPAD=NPAD, NPX=NPX)


def _split_multiwaits(nc, mybir, max_waits=1):
    """The pinned walrus codegen accepts only one sync-wait per instruction;
    move extra waits onto dedicated NoOps just before the instruction (same
    engine stream, so semantics are identical)."""
    n_split = 0
    for fn in nc.m.functions:
        for blk in fn.blocks:
            new_insts = []
            for inst in blk.instructions:
                si = getattr(inst, "sync_info", None)
                if si is not None and si.on_wait and len(si.on_wait) > max_waits:
                    waits = list(si.on_wait)
                    extra, keep = waits[:-max_waits], waits[-max_waits:]
                    for i, w in enumerate(extra):
                        new_insts.append(mybir.InstNoOp(
                            name=f"{inst.name}-w{i}",
                            sync_info=mybir.SyncInfo(on_wait=[w], on_update=[]),
                            engine=inst.engine,
                            bass_nofuse=True,
                        ))
                    inst.sync_info = mybir.SyncInfo(on_wait=keep,
                                                    on_update=si.on_update)
                    n_split += 1
                new_insts.append(inst)
            blk.instructions = new_insts
    return n_split


def _get_nc(sizes):
    sizes = tuple(sizes)
    if sizes not in _NC_CACHE:
        _NC_CACHE[sizes] = _build(sizes)
    return _NC_CACHE[sizes]


def kernel(x, batch, w_q, w_k, w_v, b_q, b_k, b_v, w_o, b_o):
    global LAST_RESULTS, LAST_NC
    x = np.asarray(x, np.float32)
    batch = np.asarray(batch, np.int64)
    counts = np.bincount(batch, minlength=NUM_GRAPHS)[:NUM_GRAPHS]
    starts = np.concatenate([[0], np.cumsum(counts)]).astype(np.int64)
    # slot k on every core gets one of the 8 graphs ranked [8k, 8k+8) by
    # size, so the shared slot width is the k-th octile max (ceil8)
    order = np.argsort(-counts, kind="stable")
    sizes = tuple(int(-(-int(counts[order[8 * k]]) // 8) * 8)
                  for k in range(GPG))
    assert all(s <= 512 for s in sizes), f"graph too large: {counts.max()}"
    nc, meta = _get_nc(sizes)
    LAST_NC = nc
    JT, OFF, NPAD, NPX = meta["JT"], meta["OFF"], meta["NPAD"], meta["NPX"]
    MB = [sum(JT[:k]) for k in range(GPG)]

    # shared host-side prepped weights (bf16 for matmul operands)
    bf16 = ml_dtypes.bfloat16
    wvT = np.zeros((DIM, VC), np.float32)
    bvp = np.zeros(VC, np.float32)
    for h in range(H):
        wvT[:, 65 * h:65 * h + 64] = w_v[64 * h:64 * h + 64, :].T
        bvp[65 * h:65 * h + 64] = b_v[64 * h:64 * h + 64]
        bvp[65 * h + 64] = 1.0
    wb = np.concatenate([w_q.T, w_k.T, wvT], axis=1)
    wb = np.ascontiguousarray(wb).astype(bf16)
    woT = w_o.T
    wo4 = np.concatenate([woT[kb * 128:(kb + 1) * 128] for kb in range(4)],
                         axis=1)
    wo4 = np.ascontiguousarray(wo4).astype(bf16)
    bvrep = np.ascontiguousarray(np.broadcast_to(bvp, (128, VC))).astype(bf16)
    misc0 = np.empty((128, 12 + sum(JT)), np.float32)
    misc0[:, 0:4] = b_q.reshape(4, 128).T
    misc0[:, 4:8] = b_k.reshape(4, 128).T
    misc0[:, 8:12] = b_o.reshape(4, 128).T

    in_maps = []
    for c in range(N_CORES):
        xs = np.zeros((NPX, DIM), np.float32)
        misc = misc0.copy()
        misc[:, 12:] = NEG
        for k in range(GPG):
            g = order[8 * k + c]
            n = int(counts[g])
            xs[OFF[k]:OFF[k] + n] = x[starts[g]:starts[g] + n]
            for jb in range(JT[k]):
                valid = np.arange(128) + jb * 128 < n
                misc[valid, 12 + MB[k] + jb] = 0.0
        in_maps.append({
            "xT": np.ascontiguousarray(xs.T).astype(bf16),
            "wb": wb, "wo4": wo4, "bvrep": bvrep, "misc": misc,
        })

    from concourse.bass_utils import run_bass_kernel_spmd
    trace = bool(os.environ.get("KTRACE"))
    LAST_RESULTS = run_bass_kernel_spmd(nc, in_maps, list(range(N_CORES)),
                                        trace=trace)

    out = np.empty((x.shape[0], DIM), np.float32)
    for c in range(N_CORES):
        oT = LAST_RESULTS.results[c]["outT"]
        for k in range(GPG):
            g = order[8 * k + c]
            n = int(counts[g])
            out[starts[g]:starts[g] + n] = oT[:, OFF[k]:OFF[k] + n].T
    return out

